# revision 8
# baseline (speedup 1.0000x reference)
"""Trainium2 Bass kernel for BaseAttentionConvolution (7x7 neighborhood attention).

Computation (reference, fp32):
    q = Q @ Wq + bq                     # [B,H,W,64]
    k = K @ Wk + bk                     # [B,H,W,64]
    S[p, (dy,dx)] = q[p] . k[p+(dy,dx)]         (7x7 window, -inf outside image)
    P = softmax(S / 8)
    O[p] = sum_j P[p,j] * V[p+j]        # [B,H,W,128]
    out = relu(O @ Wv + bv)             # [B,H,W,128]

Sharding: B*H = 192 rows split into 8 bands of 24 rows (one per core).

Fast path (bq = bk = bv = 0, the shipped configuration), bf16 matmuls:
  - Host fuses G = Wq @ Wk^T so S = x_q^T G x_k needs no q/k projections:
    kG[128, 2880] = G^T-matmul over the raw K slab (on PE), and the raw Q
    slab is the moving operand of the score matmuls directly.
  - Bands of 4 query rows; for each of the 10 k-rows of a band only the
    valid query-row range is computed (widths 1,2,3,4,4,4,4,3,2,1 x 96),
    eliminating all redundant (q-row, k-row) pairs.
  - Per band the 10 score blocks are packed into two 3-PSUM-bank tiles
    (no matmul crosses a bank) so exp and band-masking run as one big
    ACT/DVE op per phase instead of ten small ones.
  - Image-edge handling: K/V halo rows are zeros, so a halo row
    contributes exp(0)*band = band to the softmax denominator; a rank-1
    matmul subtracts the known count (-n_invalid(row) * bandwidth(x))
    from den. No kbias input, no per-row mask input.
  - den is transposed (PE) BEFORE the reciprocal so the divide runs on 96
    DVE lanes instead of 1.
  - out = relu((outT^T @ Wv) * recip) per query row; one DMA per band.

Slow path (any nonzero bias): the original f32r kernel (v1) below.
"""

import numpy as np
from contextlib import ExitStack

import ml_dtypes

import concourse.bass as bass
import concourse.bacc as bacc
import concourse.tile as tile
from concourse import mybir
from concourse.bass_utils import run_bass_kernel_spmd

DT = mybir.dt.float32
BF = mybir.dt.bfloat16
FR = mybir.dt.float32r
AF = mybir.ActivationFunctionType
BF_NP = ml_dtypes.bfloat16

# Problem constants (hardcoded per contract)
B, H, W, C, KD, OD = 2, 96, 96, 128, 64, 128
KS, PAD = 7, 3
NCORES = 8
ROWS = (B * H) // NCORES        # 24 query rows per core
KROWS = ROWS + 2 * PAD          # 30 k/v rows per core (with halo)
NQ = ROWS * W                   # 2304 query pixels per core
NK = KROWS * W                  # 2880 key pixels per core
BAND = 4                        # query rows per band
NBANDS = ROWS // BAND           # 6
BN = BAND * W                   # 384 band query columns
NKR = BAND + 2 * PAD            # 10 k-rows per band
SCALE = 1.0 / np.sqrt(KD)       # 1/8

# Per-band k-row geometry: k-row i serves query rows c in [C0[i], C0[i]+WID[i])
C0 = [max(0, i - 2 * PAD) for i in range(NKR)]
WID = [min(BAND - 1, i) - max(0, i - 2 * PAD) + 1 for i in range(NKR)]
# Packed score-tile layout: two phases of five k-rows each, 3 PSUM banks
# (1536 fp32 cols); offsets chosen so no block crosses a 512-col bank edge.
PH_I = [(0, 1, 2, 3, 4), (5, 6, 7, 8, 9)]
OFF = {0: 384, 1: 800, 2: 512, 3: 0, 4: 1024,
       5: 0, 6: 1024, 7: 512, 8: 800, 9: 384}
SPACK = 1536
SEG = ((0, 480), (512, 992), (1024, 1408))


def build_nc():
    nc = bacc.Bacc(None, target_bir_lowering=False)
    qt = nc.dram_tensor("qt", [C, NQ], BF, kind="ExternalInput")
    kt = nc.dram_tensor("kt", [C, NK], BF, kind="ExternalInput")
    v = nc.dram_tensor("v", [W, KROWS, OD], BF, kind="ExternalInput")
    gw = nc.dram_tensor("gw", [C, C + OD], BF, kind="ExternalInput")
    bandp = nc.dram_tensor("bandp", [W, SPACK], BF, kind="ExternalInput")
    wcorr = nc.dram_tensor("wcorr", [1, NBANDS * BN], BF, kind="ExternalInput")
    out = nc.dram_tensor("out", [ROWS, W, OD], DT, kind="ExternalOutput")

    with tile.TileContext(nc) as tc, ExitStack() as ctx:
        consts = ctx.enter_context(tc.tile_pool(name="consts", bufs=1))
        slabs = ctx.enter_context(tc.tile_pool(name="slabs", bufs=1))
        e_pool = ctx.enter_context(tc.tile_pool(name="e_pool", bufs=2))
        sm_pool = ctx.enter_context(tc.tile_pool(name="sm_pool", bufs=2))
        ot_pool = ctx.enter_context(tc.tile_pool(name="ot_pool", bufs=2))
        os_pool = ctx.enter_context(tc.tile_pool(name="os_pool", bufs=2))
        ps_s = ctx.enter_context(tc.tile_pool(name="ps_s", bufs=2, space="PSUM"))
        ps_o = ctx.enter_context(tc.tile_pool(name="ps_o", bufs=1, space="PSUM"))
        ps_d = ctx.enter_context(tc.tile_pool(name="ps_d", bufs=1, space="PSUM"))

        # ---- constants (gw first: kG needs it immediately) ----
        gw_s = consts.tile([C, C + OD], BF, tag="cgw")
        nc.sync.dma_start(out=gw_s[:], in_=gw[:])
        g_s = gw_s[:, :C]
        wv_s = gw_s[:, C : C + OD]
        ones1 = consts.tile([1, 1], DT, tag="cone1")
        nc.vector.memset(ones1[:], 1.0)
        oneb = consts.tile([1, 1], BF, tag="coneb")
        nc.vector.memset(oneb[:], 1.0)
        ones96 = consts.tile([W, 1], BF, tag="cones96")
        nc.vector.memset(ones96[:], 1.0)

        # ---- slabs; kt arrives in chunks so kG can start early ----
        kt_s = slabs.tile([C, NK], BF, tag="skt")
        for j0 in range(0, NK, 512):
            j1 = min(j0 + 512, NK)
            nc.sync.dma_start(out=kt_s[:, j0:j1], in_=kt[:, j0:j1])
        qt_s = slabs.tile([C, NQ], BF, tag="sqt")
        nc.sync.dma_start(out=qt_s[:], in_=qt[:])
        bandp_s = consts.tile([W, SPACK], BF, tag="cbp")
        nc.sync.dma_start(out=bandp_s[:], in_=bandp[:])
        v_s = slabs.tile([W, KROWS, OD], BF, tag="sv")
        nc.sync.dma_start(out=v_s[:], in_=v[:])
        wcorr_s = consts.tile([1, NBANDS * BN], BF, tag="cwc")
        nc.sync.dma_start(out=wcorr_s[:], in_=wcorr[:])

        # ---- kG = G^T-applied K slab: kG[:, p] = G @ k_pixel(p) ----
        kG_s = slabs.tile([C, NK], BF, tag="skG")

        def kg_chunk(j0):
            j1 = min(j0 + 512, NK)
            ps = ps_s.tile([C, 512], DT, tag="S")
            nc.tensor.matmul(
                out=ps[:, : j1 - j0], lhsT=g_s[:], rhs=kt_s[:, j0:j1],
                start=True, stop=True,
            )
            nc.scalar.copy(kG_s[:, j0:j1], ps[:, : j1 - j0])

        # ---- bands (software-pipelined: band P's tail fills band P+1's
        # exp window on PE, so the tensor engine never idles) ----
        st = [dict() for _ in range(NBANDS)]

        def tail_feed(P):
            # DVE feeders for band P's tail; emitted first so they run
            # before this band's masks occupy the vector queue.
            den_sb = sm_pool.tile([1, BN], DT, tag="densb")
            nc.vector.tensor_copy(den_sb[:], st[P]["den"][:])
            oT = ot_pool.tile([OD, BN], BF, tag="oT")
            nc.vector.tensor_copy(oT[:], st[P]["outT"][:])
            st[P]["den_sb"], st[P]["oT"] = den_sb, oT

        def tail_recip(P):
            # PE transposes of den (so the reciprocal runs on 96 lanes)
            denT = ps_o.tile([W, BAND], DT, tag="outT")
            den_sb = st[P]["den_sb"]
            for c in range(BAND):
                nc.tensor.transpose(
                    denT[:, c : c + 1], den_sb[:, c * W : (c + 1) * W], ones1[:]
                )
            recipT = sm_pool.tile([W, BAND], DT, tag="recipT")
            nc.vector.reciprocal(recipT[:], denT[:])
            st[P]["recipT"] = recipT

        def tail_out(P):
            # out-proj matmuls + relu*recip + store
            oT, recipT = st[P]["oT"], st[P]["recipT"]
            op = ps_d.tile([W, BAND * OD], DT, tag="den")
            ost = os_pool.tile([W, BAND * OD], DT, tag="ost")
            for c in range(BAND):
                nc.tensor.matmul(
                    out=op[:, c * OD : (c + 1) * OD],
                    lhsT=oT[:, c * W : (c + 1) * W],
                    rhs=wv_s[:],
                    start=True, stop=True,
                )
            for c in range(BAND):
                nc.vector.tensor_scalar(
                    ost[:, c * OD : (c + 1) * OD],
                    op[:, c * OD : (c + 1) * OD],
                    recipT[:, c : c + 1], 0.0,
                    mybir.AluOpType.mult, mybir.AluOpType.max,
                )
            h0p = P * BAND
            nc.sync.dma_start(
                out=out[h0p : h0p + BAND].rearrange("r x e -> x r e"),
                in_=ost[:].rearrange("x (r e) -> x r e", r=BAND),
            )

        for j0 in range(0, 3 * 512, 512):
            kg_chunk(j0)
        for band in range(NBANDS):
            if band < 3:
                kg_chunk((3 + band) * 512)
            h0 = band * BAND
            P = band - 1
            if P >= 0:
                tail_feed(P)
            Eph = []
            for ph in range(2):
                S = ps_s.tile([W, SPACK], DT, tag="S")
                for i in PH_I[ph]:
                    r, o, w = h0 + i, OFF[i], WID[i]
                    jq = slice((h0 + C0[i]) * W, (h0 + C0[i] + w) * W)
                    nc.tensor.matmul(
                        out=S[:, o : o + w * W],
                        lhsT=kG_s[:, r * W : (r + 1) * W],
                        rhs=qt_s[:, jq],
                        start=True, stop=True,
                    )
                E = e_pool.tile([W, SPACK], BF, tag="E")
                # per-bank segments (skips the unwritten pad columns, and
                # unblocks the first outT matmuls after just one segment)
                for s0, s1 in SEG:
                    nc.scalar.activation(
                        E[:, s0:s1], S[:, s0:s1], AF.Exp, bias=0.0, scale=SCALE
                    )
                    nc.vector.tensor_mul(
                        E[:, s0:s1], E[:, s0:s1], bandp_s[:, s0:s1]
                    )
                Eph.append(E)
                if ph == 0 and P >= 0:
                    tail_recip(P)
            if P >= 0:
                tail_out(P)
            # accumulation: the full-width i=3 block is issued first with
            # start=True so every later sub-range write is pure accumulation
            outT = ps_o.tile([OD, BN], DT, tag="outT")
            den = ps_d.tile([1, BN], DT, tag="den")
            st[band]["outT"], st[band]["den"] = outT, den
            for ph in range(2):
                E = Eph[ph]
                order = (3, 0, 1, 2, 4) if ph == 0 else PH_I[1]
                for i in order:
                    r, o, w = h0 + i, OFF[i], WID[i]
                    js = slice(C0[i] * W, (C0[i] + w) * W)
                    nc.tensor.matmul(
                        out=outT[:, js],
                        lhsT=v_s[:, r, :],
                        rhs=E[:, o : o + w * W],
                        start=(ph == 0 and i == 3), stop=(i == NKR - 1),
                    )
                for i in order:
                    r, o, w = h0 + i, OFF[i], WID[i]
                    js = slice(C0[i] * W, (C0[i] + w) * W)
                    nc.tensor.matmul(
                        out=den[:, js],
                        lhsT=ones96[:],
                        rhs=E[:, o : o + w * W],
                        start=(ph == 0 and i == 3), stop=False,
                    )
            # den -= n_invalid(row) * bandwidth(x)  (zero for interior bands)
            nc.tensor.matmul(
                out=den[:], lhsT=oneb[:],
                rhs=wcorr_s[:, band * BN : (band + 1) * BN],
                start=False, stop=True,
            )

        # final band's tail
        Pl = NBANDS - 1
        tail_feed(Pl)
        tail_recip(Pl)
        tail_out(Pl)

    nc.compile()
    return nc


def _bf(x):
    return np.ascontiguousarray(np.asarray(x, np.float32)).astype(BF_NP)


def make_in_maps(Q, K, V, Wq, bq, Wk, bk, Wv, bv):
    Q = np.asarray(Q, np.float32)
    K = np.asarray(K, np.float32)
    V = np.asarray(V, np.float32)
    G = np.asarray(Wq, np.float32) @ np.asarray(Wk, np.float32).T  # [C, C]
    gw = np.concatenate([G.T, np.asarray(Wv, np.float32)], axis=1)  # [C, C+OD]
    gwb = _bf(gw)

    # band mask constant, packed-layout [96, 1536]
    idx = np.arange(W)
    band96 = (np.abs(idx[:, None] - idx[None, :]) <= PAD).astype(np.float32)
    bandp = np.zeros((W, SPACK), np.float32)
    for i in PH_I[0]:
        o, w = OFF[i], WID[i]
        bandp[:, o : o + w * W] = np.tile(band96, (1, w))
    bandp = _bf(bandp)

    bw = (np.minimum(idx + PAD, W - 1) - np.maximum(idx - PAD, 0) + 1).astype(
        np.float32
    )  # valid kx count per x

    in_maps = []
    for core in range(NCORES):
        b = core // (H // ROWS)
        h_start = (core % (H // ROWS)) * ROWS

        qs = Q[b, h_start : h_start + ROWS]              # [24,96,128]
        qtc = _bf(qs.reshape(NQ, C).T)                   # [128,2304]

        kpad = np.zeros((KROWS, W, C), np.float32)
        vpad = np.zeros((KROWS, W, C), np.float32)
        for j in range(KROWS):
            gr = h_start - PAD + j
            if 0 <= gr < H:
                kpad[j] = K[b, gr]
                vpad[j] = V[b, gr]
        ktc = _bf(kpad.reshape(NK, C).T)                 # [128,2880]
        vtc = _bf(vpad.transpose(1, 0, 2))               # [96,30,128]

        wcorr = np.zeros((1, NBANDS * BN), np.float32)
        for band in range(NBANDS):
            for c in range(BAND):
                gr = h_start + band * BAND + c
                n_inv = sum(
                    1 for dy in range(-PAD, PAD + 1) if not (0 <= gr + dy < H)
                )
                if n_inv:
                    wcorr[0, band * BN + c * W : band * BN + (c + 1) * W] = -n_inv * bw
        in_maps.append(
            {
                "qt": qtc,
                "kt": ktc,
                "v": vtc,
                "gw": gwb,
                "bandp": bandp,
                "wcorr": _bf(wcorr),
            }
        )
    return in_maps


def gather(results):
    full = np.empty((B, H, W, OD), np.float32)
    for core in range(NCORES):
        b = core // (H // ROWS)
        h_start = (core % (H // ROWS)) * ROWS
        full[b, h_start : h_start + ROWS] = results[core]["out"]
    return full


_NC_CACHE = {}


def get_nc(path="v2"):
    if path not in _NC_CACHE:
        _NC_CACHE[path] = build_nc() if path == "v2" else build_nc_v1(
            with_bv=(path == "v1bv")
        )
    return _NC_CACHE[path]


def kernel(Q, K, V, Wq, bq, Wk, bk, Wv, bv):
    if np.any(np.asarray(bq)) or np.any(np.asarray(bk)):
        nc = get_nc("v1bv" if np.any(np.asarray(bv)) else "v1")
        in_maps = make_in_maps_v1(Q, K, V, Wq, bq, Wk, bk, Wv, bv)
    elif np.any(np.asarray(bv)):
        nc = get_nc("v1bv")
        in_maps = make_in_maps_v1(Q, K, V, Wq, bq, Wk, bk, Wv, bv)
    else:
        nc = get_nc("v2")
        in_maps = make_in_maps(Q, K, V, Wq, bq, Wk, bk, Wv, bv)
    res = run_bass_kernel_spmd(nc, in_maps, list(range(NCORES)))
    return gather(res.results)


# ======================================================================
# v1 fallback (original f32r kernel) — used only when a bias is nonzero.
# ======================================================================

WVN = 2 * OD
NEG = -30000.0


def build_nc_v1(with_bv=False):
    MDT = FR
    nc = bacc.Bacc(None, target_bir_lowering=False)
    qt = nc.dram_tensor("qt", [C, NQ], MDT, kind="ExternalInput")
    kt = nc.dram_tensor("kt", [C, NK], MDT, kind="ExternalInput")
    v = nc.dram_tensor("v", [W, KROWS, C], MDT, kind="ExternalInput")
    wq = nc.dram_tensor("wq", [C, KD], MDT, kind="ExternalInput")
    wk = nc.dram_tensor("wk", [C, KD], MDT, kind="ExternalInput")
    wv = nc.dram_tensor("wv", [C, WVN], MDT, kind="ExternalInput")
    bq = nc.dram_tensor("bq", [KD, 1], DT, kind="ExternalInput")
    bk = nc.dram_tensor("bk", [KD, 1], DT, kind="ExternalInput")
    bv = nc.dram_tensor("bv", [1, WVN], MDT, kind="ExternalInput")
    kbias = nc.dram_tensor("kbias", [W, KROWS], DT, kind="ExternalInput")
    ones_in = nc.dram_tensor("ones", [W, 1], MDT, kind="ExternalInput")
    b4 = nc.dram_tensor("b4", [W, NKR * BN], DT, kind="ExternalInput")
    out = nc.dram_tensor("out", [ROWS, W, OD], DT, kind="ExternalOutput")

    with tile.TileContext(nc) as tc, ExitStack() as ctx:
        consts = ctx.enter_context(tc.tile_pool(name="consts", bufs=1))
        slabs = ctx.enter_context(tc.tile_pool(name="slabs", bufs=1))
        e_pool = ctx.enter_context(tc.tile_pool(name="e_pool", bufs=3))
        o_pool = ctx.enter_context(tc.tile_pool(name="o_pool", bufs=2))
        r_pool = ctx.enter_context(tc.tile_pool(name="r_pool", bufs=2))
        rs_pool = ctx.enter_context(tc.tile_pool(name="rs_pool", bufs=8))
        outs = ctx.enter_context(tc.tile_pool(name="outs", bufs=3))
        ps_a = ctx.enter_context(tc.tile_pool(name="ps_a", bufs=3, space="PSUM"))
        ps_b = ctx.enter_context(tc.tile_pool(name="ps_b", bufs=2, space="PSUM"))
        ps_c = ctx.enter_context(tc.tile_pool(name="ps_c", bufs=2, space="PSUM"))

        wq_s = consts.tile([C, KD], MDT, tag="cw")
        nc.sync.dma_start(out=wq_s[:], in_=wq[:])
        wk_s = consts.tile([C, KD], MDT, tag="cw2")
        nc.sync.dma_start(out=wk_s[:], in_=wk[:])
        wv_s = consts.tile([C, WVN], MDT, tag="cw3")
        nc.sync.dma_start(out=wv_s[:], in_=wv[:])
        bq_s = consts.tile([KD, 1], DT, tag="cb")
        nc.sync.dma_start(out=bq_s[:], in_=bq[:])
        bk_s = consts.tile([KD, 1], DT, tag="cb2")
        nc.sync.dma_start(out=bk_s[:], in_=bk[:])
        kbias_s = consts.tile([W, KROWS], DT, tag="ckb")
        nc.sync.dma_start(out=kbias_s[:], in_=kbias[:])
        b4_s = consts.tile([W, NKR * BN], DT, tag="cb4")
        nc.sync.dma_start(out=b4_s[:], in_=b4[:])
        ones96 = consts.tile([W, 1], MDT, tag="cones")
        nc.sync.dma_start(out=ones96[:], in_=ones_in[:])
        ones1 = consts.tile([1, 1], DT, tag="cone1")
        nc.vector.memset(ones1[:], 1.0)
        if with_bv:
            bv_s = consts.tile([1, WVN], MDT, tag="cbv")
            nc.sync.dma_start(out=bv_s[:], in_=bv[:])

        qt_s = slabs.tile([C, NQ], MDT, tag="sqt")
        nc.sync.dma_start(out=qt_s[:], in_=qt[:])
        kt_s = slabs.tile([C, NK], MDT, tag="skt")
        nc.sync.dma_start(out=kt_s[:], in_=kt[:])
        v_s = slabs.tile([W, KROWS, C], MDT, tag="sv")
        nc.sync.dma_start(out=v_s[:], in_=v[:])

        qT_s = slabs.tile([KD, NQ], MDT, tag="sqT")
        kT_s = slabs.tile([KD, NK], MDT, tag="skT")
        for dst, src, wmat, bvec, n in (
            (qT_s, qt_s, wq_s, bq_s, NQ),
            (kT_s, kt_s, wk_s, bk_s, NK),
        ):
            for j0 in range(0, n, 512):
                j1 = min(j0 + 512, n)
                ps = ps_a.tile([KD, 512], DT, tag="w")
                nc.tensor.matmul(
                    out=ps[:, : j1 - j0], lhsT=wmat[:], rhs=src[:, j0:j1],
                    start=True, stop=True,
                )
                nc.scalar.activation(
                    dst[:, j0:j1], ps[:, : j1 - j0], AF.Identity,
                    bias=bvec[:], scale=1.0,
                )

        for band in range(NBANDS):
            h0 = band * BAND
            jq = slice(h0 * W, (h0 + BAND) * W)
            outT = ps_b.tile([OD, BN], DT, tag="outT")
            den = ps_c.tile([1, BN], DT, tag="den")
            for i in range(NKR):
                r = h0 + i
                S = ps_a.tile([W, BN], DT, tag="w")
                nc.tensor.matmul(
                    out=S[:], lhsT=kT_s[:, r * W : (r + 1) * W],
                    rhs=qT_s[:, jq], start=True, stop=True,
                )
                E = e_pool.tile([W, BN], MDT, tag="E")
                nc.scalar.activation(
                    E[:], S[:], AF.Exp, bias=kbias_s[:, r : r + 1], scale=SCALE
                )
                nc.vector.tensor_mul(E[:], E[:], b4_s[:, i * BN : (i + 1) * BN])
                nc.tensor.matmul(
                    out=outT[:], lhsT=v_s[:, r, :], rhs=E[:],
                    start=(i == 0), stop=(i == NKR - 1),
                )
                nc.tensor.matmul(
                    out=den[:], lhsT=ones96[:], rhs=E[:],
                    start=(i == 0), stop=(i == NKR - 1),
                )

            recip = r_pool.tile([1, BN], DT, tag="recip")
            nc.vector.reciprocal(recip[:], den[:])
            oT = o_pool.tile([OD, BN], MDT, tag="oT")
            nc.vector.tensor_copy(oT[:], outT[:])
            if with_bv:
                den_sb = r_pool.tile([1, BN], MDT, tag="densb")
                nc.vector.tensor_copy(den_sb[:], den[:])
            for c in range(BAND):
                cs = slice(c * W, (c + 1) * W)
                rT = ps_a.tile([W, 1], DT, tag="w")
                nc.tensor.transpose(rT[:], recip[:, cs], ones1[:])
                rS = rs_pool.tile([W, 1], DT, tag="rS")
                nc.vector.tensor_copy(rS[:], rT[:])
                op = ps_a.tile([W, WVN], DT, tag="w")
                nc.tensor.matmul(
                    out=op[:], lhsT=oT[:, cs], rhs=wv_s[:],
                    start=True, stop=not with_bv,
                )
                if with_bv:
                    nc.tensor.matmul(
                        out=op[:], lhsT=den_sb[:, cs], rhs=bv_s[:],
                        start=False, stop=True,
                    )
                ost = outs.tile([W, OD], DT, tag="ost")
                nc.scalar.activation(ost[:], op[:, :OD], AF.Relu, bias=0.0, scale=rS[:])
                nc.sync.dma_start(out=out[h0 + c], in_=ost[:])

    nc.compile()
    return nc


def round_f32r(x):
    b = np.ascontiguousarray(x, np.float32).view(np.uint32)
    tie = (b >> 12) & 1
    b = (b + 0x7FF + tie) & np.uint32(0xFFFFF000)
    return b.view(np.float32)


def make_in_maps_v1(Q, K, V, Wq, bq, Wk, bk, Wv, bv):
    rnd = round_f32r
    Q = np.asarray(Q, np.float32)
    K = np.asarray(K, np.float32)
    V = np.asarray(V, np.float32)
    Wqr = rnd(np.asarray(Wq, np.float32))
    Wkr = rnd(np.asarray(Wk, np.float32))
    wvp = np.zeros((C, WVN), np.float32)
    wvp[:, :OD] = np.asarray(Wv, np.float32)
    wvp = rnd(wvp)
    bqv = np.ascontiguousarray(np.asarray(bq, np.float32).reshape(KD, 1))
    bkv = np.ascontiguousarray(np.asarray(bk, np.float32).reshape(KD, 1))
    bvp = np.zeros((1, WVN), np.float32)
    bvp[0, :OD] = np.asarray(bv, np.float32)
    bvp = rnd(bvp)

    idx = np.arange(W)
    b4 = (np.abs(idx[:, None] - idx[None, :]) <= PAD).astype(np.float32)
    b4i = np.zeros((W, NKR, BAND, W), np.float32)
    for i in range(NKR):
        for c in range(BAND):
            if i - 2 * PAD <= c <= i:
                b4i[:, i, c, :] = b4
    b4rep = np.ascontiguousarray(b4i.reshape(W, NKR * BAND * W))

    in_maps = []
    for core in range(NCORES):
        b = core // (H // ROWS)
        h_start = (core % (H // ROWS)) * ROWS
        qs = Q[b, h_start : h_start + ROWS]
        qtc = rnd(np.ascontiguousarray(qs.reshape(NQ, C).T))
        kpad = np.zeros((KROWS, W, C), np.float32)
        vpad = np.zeros((KROWS, W, C), np.float32)
        kb = np.full((KROWS,), NEG, np.float32)
        for j in range(KROWS):
            gr = h_start - PAD + j
            if 0 <= gr < H:
                kpad[j] = K[b, gr]
                vpad[j] = V[b, gr]
                kb[j] = 0.0
        ktc = rnd(np.ascontiguousarray(kpad.reshape(NK, C).T))
        vtc = rnd(np.ascontiguousarray(vpad.transpose(1, 0, 2)))
        kbias = np.ascontiguousarray(np.broadcast_to(kb[None, :], (W, KROWS)))
        in_maps.append(
            {
                "qt": qtc, "kt": ktc, "v": vtc,
                "wq": Wqr, "wk": Wkr, "wv": wvp,
                "bq": bqv, "bk": bkv, "bv": bvp,
                "kbias": kbias,
                "ones": np.ones((W, 1), np.float32),
                "b4": b4rep,
            }
        )
    return in_maps


# revision 9
# speedup vs baseline: 1.1635x; 1.1635x over previous
"""Trainium2 Bass kernel for BaseAttentionConvolution (7x7 neighborhood attention).

Computation (reference, fp32):
    q = Q @ Wq + bq                     # [B,H,W,64]
    k = K @ Wk + bk                     # [B,H,W,64]
    S[p, (dy,dx)] = q[p] . k[p+(dy,dx)]         (7x7 window, -inf outside image)
    P = softmax(S / 8)
    O[p] = sum_j P[p,j] * V[p+j]        # [B,H,W,128]
    out = relu(O @ Wv + bv)             # [B,H,W,128]

Sharding: B*H = 192 rows split into 8 bands of 24 rows (one per core).

Fast path (bq = bk = bv = 0, the shipped configuration), bf16 matmuls:
  - Host fuses G = Wq @ Wk^T so S = x_q^T G x_k needs no q/k projections:
    kG[128, 2880] = G^T-matmul over the raw K slab (on PE), and the raw Q
    slab is the moving operand of the score matmuls directly.
  - Bands of 4 query rows; for each of the 10 k-rows of a band only the
    valid query-row range is computed (widths 1,2,3,4,4,4,4,3,2,1 x 96),
    eliminating all redundant (q-row, k-row) pairs.
  - Per band the 10 score blocks are packed into two 3-PSUM-bank tiles
    (no matmul crosses a bank) so exp and band-masking run as one big
    ACT/DVE op per phase instead of ten small ones.
  - Image-edge handling: K/V halo rows are zeros, so a halo row
    contributes exp(0)*band = band to the softmax denominator; a rank-1
    matmul subtracts the known count (-n_invalid(row) * bandwidth(x))
    from den. No kbias input, no per-row mask input.
  - den is transposed (PE) BEFORE the reciprocal so the divide runs on 96
    DVE lanes instead of 1.
  - out = relu((outT^T @ Wv) * recip) per query row; one DMA per band.

Slow path (any nonzero bias): the original f32r kernel (v1) below.
"""

import numpy as np
from contextlib import ExitStack

import ml_dtypes

import concourse.bass as bass
import concourse.bacc as bacc
import concourse.tile as tile
from concourse import mybir
from concourse.bass_utils import run_bass_kernel_spmd

DT = mybir.dt.float32
BF = mybir.dt.bfloat16
FR = mybir.dt.float32r
AF = mybir.ActivationFunctionType
BF_NP = ml_dtypes.bfloat16

# Problem constants (hardcoded per contract)
B, H, W, C, KD, OD = 2, 96, 96, 128, 64, 128
KS, PAD = 7, 3
NCORES = 8
ROWS = (B * H) // NCORES        # 24 query rows per core
KROWS = ROWS + 2 * PAD          # 30 k/v rows per core (with halo)
NQ = ROWS * W                   # 2304 query pixels per core
NK = KROWS * W                  # 2880 key pixels per core
BAND = 4                        # query rows per band
NBANDS = ROWS // BAND           # 6
BN = BAND * W                   # 384 band query columns
NKR = BAND + 2 * PAD            # 10 k-rows per band
SCALE = 1.0 / np.sqrt(KD)       # 1/8

# Per-band k-row geometry: k-row i serves query rows c in [C0[i], C0[i]+WID[i])
C0 = [max(0, i - 2 * PAD) for i in range(NKR)]
WID = [min(BAND - 1, i) - max(0, i - 2 * PAD) + 1 for i in range(NKR)]
# Packed score-tile layout: two phases of five k-rows each, 3 PSUM banks
# (1536 fp32 cols); offsets chosen so no block crosses a 512-col bank edge.
PH_I = [(0, 1, 2, 3, 4), (5, 6, 7, 8, 9)]
OFF = {0: 384, 1: 800, 2: 512, 3: 0, 4: 1024,
       5: 0, 6: 1024, 7: 512, 8: 800, 9: 384}
SPACK = 1536
SEG = ((0, 480), (512, 992), (1024, 1408))


def build_nc():
    nc = bacc.Bacc(None, target_bir_lowering=False)
    qt = nc.dram_tensor("qt", [C, NQ], BF, kind="ExternalInput")
    kt = nc.dram_tensor("kt", [C, NK], BF, kind="ExternalInput")
    v = nc.dram_tensor("v", [W, KROWS, OD], BF, kind="ExternalInput")
    gw = nc.dram_tensor("gw", [C, C + OD], BF, kind="ExternalInput")
    bandp = nc.dram_tensor("bandp", [W, SPACK], BF, kind="ExternalInput")
    wcorr = nc.dram_tensor("wcorr", [1, NBANDS * BN], BF, kind="ExternalInput")
    out = nc.dram_tensor("out", [ROWS, W, OD], DT, kind="ExternalOutput")

    with tile.TileContext(nc) as tc, ExitStack() as ctx:
        consts = ctx.enter_context(tc.tile_pool(name="consts", bufs=1))
        slabs = ctx.enter_context(tc.tile_pool(name="slabs", bufs=1))
        e_pool = ctx.enter_context(tc.tile_pool(name="e_pool", bufs=2))
        sm_pool = ctx.enter_context(tc.tile_pool(name="sm_pool", bufs=2))
        ot_pool = ctx.enter_context(tc.tile_pool(name="ot_pool", bufs=2))
        os_pool = ctx.enter_context(tc.tile_pool(name="os_pool", bufs=2))
        ps_s = ctx.enter_context(tc.tile_pool(name="ps_s", bufs=2, space="PSUM"))
        ps_o = ctx.enter_context(tc.tile_pool(name="ps_o", bufs=1, space="PSUM"))
        ps_d = ctx.enter_context(tc.tile_pool(name="ps_d", bufs=1, space="PSUM"))

        # ---- constants (gw first: kG needs it immediately) ----
        gw_s = consts.tile([C, C + OD], BF, tag="cgw")
        nc.sync.dma_start(out=gw_s[:], in_=gw[:])
        g_s = gw_s[:, :C]
        wv_s = gw_s[:, C : C + OD]
        ones1 = consts.tile([1, 1], DT, tag="cone1")
        nc.vector.memset(ones1[:], 1.0)
        oneb = consts.tile([1, 1], BF, tag="coneb")
        nc.vector.memset(oneb[:], 1.0)
        ones96 = consts.tile([W, 1], BF, tag="cones96")
        nc.vector.memset(ones96[:], 1.0)

        # ---- slabs; kt arrives in chunks so kG can start early ----
        kt_s = slabs.tile([C, NK], BF, tag="skt")
        for j0 in range(0, NK, 512):
            j1 = min(j0 + 512, NK)
            nc.sync.dma_start(out=kt_s[:, j0:j1], in_=kt[:, j0:j1])
        qt_s = slabs.tile([C, NQ], BF, tag="sqt")
        nc.sync.dma_start(out=qt_s[:], in_=qt[:])
        bandp_s = consts.tile([W, SPACK], BF, tag="cbp")
        nc.sync.dma_start(out=bandp_s[:], in_=bandp[:])
        v_s = slabs.tile([W, KROWS, OD], BF, tag="sv")
        nc.sync.dma_start(out=v_s[:], in_=v[:])
        wcorr_s = consts.tile([1, NBANDS * BN], BF, tag="cwc")
        nc.sync.dma_start(out=wcorr_s[:], in_=wcorr[:])

        # ---- kG = G^T-applied K slab: kG[:, p] = G @ k_pixel(p) ----
        kG_s = slabs.tile([C, NK], BF, tag="skG")

        def kg_chunk(j0):
            j1 = min(j0 + 512, NK)
            ps = ps_s.tile([C, 512], DT, tag="S")
            nc.tensor.matmul(
                out=ps[:, : j1 - j0], lhsT=g_s[:], rhs=kt_s[:, j0:j1],
                start=True, stop=True,
            )
            nc.scalar.copy(kG_s[:, j0:j1], ps[:, : j1 - j0])

        # ---- bands (software-pipelined: band P's tail fills band P+1's
        # exp window on PE, so the tensor engine never idles) ----
        st = [dict() for _ in range(NBANDS)]

        def tail_feed(P):
            # DVE feeders for band P's tail; emitted first so they run
            # before this band's masks occupy the vector queue.
            den_sb = sm_pool.tile([1, BN], DT, tag="densb")
            nc.vector.tensor_copy(den_sb[:], st[P]["den"][:])
            oT = ot_pool.tile([OD, BN], BF, tag="oT")
            nc.vector.tensor_copy(oT[:], st[P]["outT"][:])
            st[P]["den_sb"], st[P]["oT"] = den_sb, oT

        def tail_recip(P):
            # PE transposes of den (so the reciprocal runs on 96 lanes)
            denT = ps_o.tile([W, BAND], DT, tag="outT")
            den_sb = st[P]["den_sb"]
            for c in range(BAND):
                nc.tensor.transpose(
                    denT[:, c : c + 1], den_sb[:, c * W : (c + 1) * W], ones1[:]
                )
            recipT = sm_pool.tile([W, BAND], DT, tag="recipT")
            nc.vector.reciprocal(recipT[:], denT[:])
            st[P]["recipT"] = recipT

        def tail_out(P):
            # out-proj matmuls + relu*recip + store
            oT, recipT = st[P]["oT"], st[P]["recipT"]
            op = ps_d.tile([W, BAND * OD], DT, tag="den")
            ost = os_pool.tile([W, BAND * OD], DT, tag="ost")
            for c in range(BAND):
                nc.tensor.matmul(
                    out=op[:, c * OD : (c + 1) * OD],
                    lhsT=oT[:, c * W : (c + 1) * W],
                    rhs=wv_s[:],
                    start=True, stop=True,
                )
            for c in range(BAND):
                nc.vector.tensor_scalar(
                    ost[:, c * OD : (c + 1) * OD],
                    op[:, c * OD : (c + 1) * OD],
                    recipT[:, c : c + 1], 0.0,
                    mybir.AluOpType.mult, mybir.AluOpType.max,
                )
            h0p = P * BAND
            nc.sync.dma_start(
                out=out[h0p : h0p + BAND].rearrange("r x e -> x r e"),
                in_=ost[:].rearrange("x (r e) -> x r e", r=BAND),
            )

        for j0 in range(0, 3 * 512, 512):
            kg_chunk(j0)
        for band in range(NBANDS):
            if band < 3:
                kg_chunk((3 + band) * 512)
            h0 = band * BAND
            P = band - 1
            if P >= 0:
                tail_feed(P)
            Eph = []
            for ph in range(2):
                S = ps_s.tile([W, SPACK], DT, tag="S")
                for p0, p1 in ((480, 512), (992, 1024), (1408, 1536)):
                    nc.vector.memset(S[:, p0:p1], 0.0)
                for i in PH_I[ph]:
                    r, o, w = h0 + i, OFF[i], WID[i]
                    jq = slice((h0 + C0[i]) * W, (h0 + C0[i] + w) * W)
                    nc.tensor.matmul(
                        out=S[:, o : o + w * W],
                        lhsT=kG_s[:, r * W : (r + 1) * W],
                        rhs=qt_s[:, jq],
                        start=True, stop=True,
                    )
                E = e_pool.tile([W, SPACK], BF, tag="E")
                nc.scalar.activation(E[:], S[:], AF.Exp, bias=0.0, scale=SCALE)
                nc.vector.tensor_mul(E[:], E[:], bandp_s[:])
                Eph.append(E)
                if ph == 0 and P >= 0:
                    tail_recip(P)
            if P >= 0:
                tail_out(P)
            # accumulation: the full-width i=3 block is issued first with
            # start=True so every later sub-range write is pure accumulation
            outT = ps_o.tile([OD, BN], DT, tag="outT")
            den = ps_d.tile([1, BN], DT, tag="den")
            st[band]["outT"], st[band]["den"] = outT, den
            for ph in range(2):
                E = Eph[ph]
                order = (3, 0, 1, 2, 4) if ph == 0 else PH_I[1]
                for i in order:
                    r, o, w = h0 + i, OFF[i], WID[i]
                    js = slice(C0[i] * W, (C0[i] + w) * W)
                    nc.tensor.matmul(
                        out=outT[:, js],
                        lhsT=v_s[:, r, :],
                        rhs=E[:, o : o + w * W],
                        start=(ph == 0 and i == 3), stop=(i == NKR - 1),
                    )
                for i in order:
                    r, o, w = h0 + i, OFF[i], WID[i]
                    js = slice(C0[i] * W, (C0[i] + w) * W)
                    nc.tensor.matmul(
                        out=den[:, js],
                        lhsT=ones96[:],
                        rhs=E[:, o : o + w * W],
                        start=(ph == 0 and i == 3), stop=False,
                    )
            # den -= n_invalid(row) * bandwidth(x)  (zero for interior bands)
            nc.tensor.matmul(
                out=den[:], lhsT=oneb[:],
                rhs=wcorr_s[:, band * BN : (band + 1) * BN],
                start=False, stop=True,
            )

        # final band's tail
        Pl = NBANDS - 1
        tail_feed(Pl)
        tail_recip(Pl)
        tail_out(Pl)

    nc.compile()
    return nc


def _bf(x):
    return np.ascontiguousarray(np.asarray(x, np.float32)).astype(BF_NP)


def make_in_maps(Q, K, V, Wq, bq, Wk, bk, Wv, bv):
    Q = np.asarray(Q, np.float32)
    K = np.asarray(K, np.float32)
    V = np.asarray(V, np.float32)
    G = np.asarray(Wq, np.float32) @ np.asarray(Wk, np.float32).T  # [C, C]
    gw = np.concatenate([G.T, np.asarray(Wv, np.float32)], axis=1)  # [C, C+OD]
    gwb = _bf(gw)

    # band mask constant, packed-layout [96, 1536]
    idx = np.arange(W)
    band96 = (np.abs(idx[:, None] - idx[None, :]) <= PAD).astype(np.float32)
    bandp = np.zeros((W, SPACK), np.float32)
    for i in PH_I[0]:
        o, w = OFF[i], WID[i]
        bandp[:, o : o + w * W] = np.tile(band96, (1, w))
    bandp = _bf(bandp)

    bw = (np.minimum(idx + PAD, W - 1) - np.maximum(idx - PAD, 0) + 1).astype(
        np.float32
    )  # valid kx count per x

    in_maps = []
    for core in range(NCORES):
        b = core // (H // ROWS)
        h_start = (core % (H // ROWS)) * ROWS

        qs = Q[b, h_start : h_start + ROWS]              # [24,96,128]
        qtc = _bf(qs.reshape(NQ, C).T)                   # [128,2304]

        kpad = np.zeros((KROWS, W, C), np.float32)
        vpad = np.zeros((KROWS, W, C), np.float32)
        for j in range(KROWS):
            gr = h_start - PAD + j
            if 0 <= gr < H:
                kpad[j] = K[b, gr]
                vpad[j] = V[b, gr]
        ktc = _bf(kpad.reshape(NK, C).T)                 # [128,2880]
        vtc = _bf(vpad.transpose(1, 0, 2))               # [96,30,128]

        wcorr = np.zeros((1, NBANDS * BN), np.float32)
        for band in range(NBANDS):
            for c in range(BAND):
                gr = h_start + band * BAND + c
                n_inv = sum(
                    1 for dy in range(-PAD, PAD + 1) if not (0 <= gr + dy < H)
                )
                if n_inv:
                    wcorr[0, band * BN + c * W : band * BN + (c + 1) * W] = -n_inv * bw
        in_maps.append(
            {
                "qt": qtc,
                "kt": ktc,
                "v": vtc,
                "gw": gwb,
                "bandp": bandp,
                "wcorr": _bf(wcorr),
            }
        )
    return in_maps


def gather(results):
    full = np.empty((B, H, W, OD), np.float32)
    for core in range(NCORES):
        b = core // (H // ROWS)
        h_start = (core % (H // ROWS)) * ROWS
        full[b, h_start : h_start + ROWS] = results[core]["out"]
    return full


_NC_CACHE = {}


def get_nc(path="v2"):
    if path not in _NC_CACHE:
        _NC_CACHE[path] = build_nc() if path == "v2" else build_nc_v1(
            with_bv=(path == "v1bv")
        )
    return _NC_CACHE[path]


def kernel(Q, K, V, Wq, bq, Wk, bk, Wv, bv):
    if np.any(np.asarray(bq)) or np.any(np.asarray(bk)):
        nc = get_nc("v1bv" if np.any(np.asarray(bv)) else "v1")
        in_maps = make_in_maps_v1(Q, K, V, Wq, bq, Wk, bk, Wv, bv)
    elif np.any(np.asarray(bv)):
        nc = get_nc("v1bv")
        in_maps = make_in_maps_v1(Q, K, V, Wq, bq, Wk, bk, Wv, bv)
    else:
        nc = get_nc("v2")
        in_maps = make_in_maps(Q, K, V, Wq, bq, Wk, bk, Wv, bv)
    res = run_bass_kernel_spmd(nc, in_maps, list(range(NCORES)))
    return gather(res.results)


# ======================================================================
# v1 fallback (original f32r kernel) — used only when a bias is nonzero.
# ======================================================================

WVN = 2 * OD
NEG = -30000.0


def build_nc_v1(with_bv=False):
    MDT = FR
    nc = bacc.Bacc(None, target_bir_lowering=False)
    qt = nc.dram_tensor("qt", [C, NQ], MDT, kind="ExternalInput")
    kt = nc.dram_tensor("kt", [C, NK], MDT, kind="ExternalInput")
    v = nc.dram_tensor("v", [W, KROWS, C], MDT, kind="ExternalInput")
    wq = nc.dram_tensor("wq", [C, KD], MDT, kind="ExternalInput")
    wk = nc.dram_tensor("wk", [C, KD], MDT, kind="ExternalInput")
    wv = nc.dram_tensor("wv", [C, WVN], MDT, kind="ExternalInput")
    bq = nc.dram_tensor("bq", [KD, 1], DT, kind="ExternalInput")
    bk = nc.dram_tensor("bk", [KD, 1], DT, kind="ExternalInput")
    bv = nc.dram_tensor("bv", [1, WVN], MDT, kind="ExternalInput")
    kbias = nc.dram_tensor("kbias", [W, KROWS], DT, kind="ExternalInput")
    ones_in = nc.dram_tensor("ones", [W, 1], MDT, kind="ExternalInput")
    b4 = nc.dram_tensor("b4", [W, NKR * BN], DT, kind="ExternalInput")
    out = nc.dram_tensor("out", [ROWS, W, OD], DT, kind="ExternalOutput")

    with tile.TileContext(nc) as tc, ExitStack() as ctx:
        consts = ctx.enter_context(tc.tile_pool(name="consts", bufs=1))
        slabs = ctx.enter_context(tc.tile_pool(name="slabs", bufs=1))
        e_pool = ctx.enter_context(tc.tile_pool(name="e_pool", bufs=3))
        o_pool = ctx.enter_context(tc.tile_pool(name="o_pool", bufs=2))
        r_pool = ctx.enter_context(tc.tile_pool(name="r_pool", bufs=2))
        rs_pool = ctx.enter_context(tc.tile_pool(name="rs_pool", bufs=8))
        outs = ctx.enter_context(tc.tile_pool(name="outs", bufs=3))
        ps_a = ctx.enter_context(tc.tile_pool(name="ps_a", bufs=3, space="PSUM"))
        ps_b = ctx.enter_context(tc.tile_pool(name="ps_b", bufs=2, space="PSUM"))
        ps_c = ctx.enter_context(tc.tile_pool(name="ps_c", bufs=2, space="PSUM"))

        wq_s = consts.tile([C, KD], MDT, tag="cw")
        nc.sync.dma_start(out=wq_s[:], in_=wq[:])
        wk_s = consts.tile([C, KD], MDT, tag="cw2")
        nc.sync.dma_start(out=wk_s[:], in_=wk[:])
        wv_s = consts.tile([C, WVN], MDT, tag="cw3")
        nc.sync.dma_start(out=wv_s[:], in_=wv[:])
        bq_s = consts.tile([KD, 1], DT, tag="cb")
        nc.sync.dma_start(out=bq_s[:], in_=bq[:])
        bk_s = consts.tile([KD, 1], DT, tag="cb2")
        nc.sync.dma_start(out=bk_s[:], in_=bk[:])
        kbias_s = consts.tile([W, KROWS], DT, tag="ckb")
        nc.sync.dma_start(out=kbias_s[:], in_=kbias[:])
        b4_s = consts.tile([W, NKR * BN], DT, tag="cb4")
        nc.sync.dma_start(out=b4_s[:], in_=b4[:])
        ones96 = consts.tile([W, 1], MDT, tag="cones")
        nc.sync.dma_start(out=ones96[:], in_=ones_in[:])
        ones1 = consts.tile([1, 1], DT, tag="cone1")
        nc.vector.memset(ones1[:], 1.0)
        if with_bv:
            bv_s = consts.tile([1, WVN], MDT, tag="cbv")
            nc.sync.dma_start(out=bv_s[:], in_=bv[:])

        qt_s = slabs.tile([C, NQ], MDT, tag="sqt")
        nc.sync.dma_start(out=qt_s[:], in_=qt[:])
        kt_s = slabs.tile([C, NK], MDT, tag="skt")
        nc.sync.dma_start(out=kt_s[:], in_=kt[:])
        v_s = slabs.tile([W, KROWS, C], MDT, tag="sv")
        nc.sync.dma_start(out=v_s[:], in_=v[:])

        qT_s = slabs.tile([KD, NQ], MDT, tag="sqT")
        kT_s = slabs.tile([KD, NK], MDT, tag="skT")
        for dst, src, wmat, bvec, n in (
            (qT_s, qt_s, wq_s, bq_s, NQ),
            (kT_s, kt_s, wk_s, bk_s, NK),
        ):
            for j0 in range(0, n, 512):
                j1 = min(j0 + 512, n)
                ps = ps_a.tile([KD, 512], DT, tag="w")
                nc.tensor.matmul(
                    out=ps[:, : j1 - j0], lhsT=wmat[:], rhs=src[:, j0:j1],
                    start=True, stop=True,
                )
                nc.scalar.activation(
                    dst[:, j0:j1], ps[:, : j1 - j0], AF.Identity,
                    bias=bvec[:], scale=1.0,
                )

        for band in range(NBANDS):
            h0 = band * BAND
            jq = slice(h0 * W, (h0 + BAND) * W)
            outT = ps_b.tile([OD, BN], DT, tag="outT")
            den = ps_c.tile([1, BN], DT, tag="den")
            for i in range(NKR):
                r = h0 + i
                S = ps_a.tile([W, BN], DT, tag="w")
                nc.tensor.matmul(
                    out=S[:], lhsT=kT_s[:, r * W : (r + 1) * W],
                    rhs=qT_s[:, jq], start=True, stop=True,
                )
                E = e_pool.tile([W, BN], MDT, tag="E")
                nc.scalar.activation(
                    E[:], S[:], AF.Exp, bias=kbias_s[:, r : r + 1], scale=SCALE
                )
                nc.vector.tensor_mul(E[:], E[:], b4_s[:, i * BN : (i + 1) * BN])
                nc.tensor.matmul(
                    out=outT[:], lhsT=v_s[:, r, :], rhs=E[:],
                    start=(i == 0), stop=(i == NKR - 1),
                )
                nc.tensor.matmul(
                    out=den[:], lhsT=ones96[:], rhs=E[:],
                    start=(i == 0), stop=(i == NKR - 1),
                )

            recip = r_pool.tile([1, BN], DT, tag="recip")
            nc.vector.reciprocal(recip[:], den[:])
            oT = o_pool.tile([OD, BN], MDT, tag="oT")
            nc.vector.tensor_copy(oT[:], outT[:])
            if with_bv:
                den_sb = r_pool.tile([1, BN], MDT, tag="densb")
                nc.vector.tensor_copy(den_sb[:], den[:])
            for c in range(BAND):
                cs = slice(c * W, (c + 1) * W)
                rT = ps_a.tile([W, 1], DT, tag="w")
                nc.tensor.transpose(rT[:], recip[:, cs], ones1[:])
                rS = rs_pool.tile([W, 1], DT, tag="rS")
                nc.vector.tensor_copy(rS[:], rT[:])
                op = ps_a.tile([W, WVN], DT, tag="w")
                nc.tensor.matmul(
                    out=op[:], lhsT=oT[:, cs], rhs=wv_s[:],
                    start=True, stop=not with_bv,
                )
                if with_bv:
                    nc.tensor.matmul(
                        out=op[:], lhsT=den_sb[:, cs], rhs=bv_s[:],
                        start=False, stop=True,
                    )
                ost = outs.tile([W, OD], DT, tag="ost")
                nc.scalar.activation(ost[:], op[:, :OD], AF.Relu, bias=0.0, scale=rS[:])
                nc.sync.dma_start(out=out[h0 + c], in_=ost[:])

    nc.compile()
    return nc


def round_f32r(x):
    b = np.ascontiguousarray(x, np.float32).view(np.uint32)
    tie = (b >> 12) & 1
    b = (b + 0x7FF + tie) & np.uint32(0xFFFFF000)
    return b.view(np.float32)


def make_in_maps_v1(Q, K, V, Wq, bq, Wk, bk, Wv, bv):
    rnd = round_f32r
    Q = np.asarray(Q, np.float32)
    K = np.asarray(K, np.float32)
    V = np.asarray(V, np.float32)
    Wqr = rnd(np.asarray(Wq, np.float32))
    Wkr = rnd(np.asarray(Wk, np.float32))
    wvp = np.zeros((C, WVN), np.float32)
    wvp[:, :OD] = np.asarray(Wv, np.float32)
    wvp = rnd(wvp)
    bqv = np.ascontiguousarray(np.asarray(bq, np.float32).reshape(KD, 1))
    bkv = np.ascontiguousarray(np.asarray(bk, np.float32).reshape(KD, 1))
    bvp = np.zeros((1, WVN), np.float32)
    bvp[0, :OD] = np.asarray(bv, np.float32)
    bvp = rnd(bvp)

    idx = np.arange(W)
    b4 = (np.abs(idx[:, None] - idx[None, :]) <= PAD).astype(np.float32)
    b4i = np.zeros((W, NKR, BAND, W), np.float32)
    for i in range(NKR):
        for c in range(BAND):
            if i - 2 * PAD <= c <= i:
                b4i[:, i, c, :] = b4
    b4rep = np.ascontiguousarray(b4i.reshape(W, NKR * BAND * W))

    in_maps = []
    for core in range(NCORES):
        b = core // (H // ROWS)
        h_start = (core % (H // ROWS)) * ROWS
        qs = Q[b, h_start : h_start + ROWS]
        qtc = rnd(np.ascontiguousarray(qs.reshape(NQ, C).T))
        kpad = np.zeros((KROWS, W, C), np.float32)
        vpad = np.zeros((KROWS, W, C), np.float32)
        kb = np.full((KROWS,), NEG, np.float32)
        for j in range(KROWS):
            gr = h_start - PAD + j
            if 0 <= gr < H:
                kpad[j] = K[b, gr]
                vpad[j] = V[b, gr]
                kb[j] = 0.0
        ktc = rnd(np.ascontiguousarray(kpad.reshape(NK, C).T))
        vtc = rnd(np.ascontiguousarray(vpad.transpose(1, 0, 2)))
        kbias = np.ascontiguousarray(np.broadcast_to(kb[None, :], (W, KROWS)))
        in_maps.append(
            {
                "qt": qtc, "kt": ktc, "v": vtc,
                "wq": Wqr, "wk": Wkr, "wv": wvp,
                "bq": bqv, "bk": bkv, "bv": bvp,
                "kbias": kbias,
                "ones": np.ones((W, 1), np.float32),
                "b4": b4rep,
            }
        )
    return in_maps


# revision 10
# speedup vs baseline: 1.2199x; 1.0484x over previous
"""Trainium2 Bass kernel for BaseAttentionConvolution (7x7 neighborhood attention).

Computation (reference, fp32):
    q = Q @ Wq + bq                     # [B,H,W,64]
    k = K @ Wk + bk                     # [B,H,W,64]
    S[p, (dy,dx)] = q[p] . k[p+(dy,dx)]         (7x7 window, -inf outside image)
    P = softmax(S / 8)
    O[p] = sum_j P[p,j] * V[p+j]        # [B,H,W,128]
    out = relu(O @ Wv + bv)             # [B,H,W,128]

Sharding: B*H = 192 rows split into 8 bands of 24 rows (one per core).

Fast path (bq = bk = bv = 0, the shipped configuration), bf16 matmuls:
  - Host fuses G = Wq @ Wk^T so S = x_q^T G x_k needs no q/k projections:
    kG[128, 2880] = G^T-matmul over the raw K slab (on PE), and the raw Q
    slab is the moving operand of the score matmuls directly.
  - Bands of 4 query rows; for each of the 10 k-rows of a band only the
    valid query-row range is computed (widths 1,2,3,4,4,4,4,3,2,1 x 96),
    eliminating all redundant (q-row, k-row) pairs.
  - Per band the 10 score blocks are packed into two 3-PSUM-bank tiles
    (no matmul crosses a bank) so exp and band-masking run as one big
    ACT/DVE op per phase instead of ten small ones.
  - Image-edge handling: K/V halo rows are zeros, so a halo row
    contributes exp(0)*band = band to the softmax denominator; a rank-1
    matmul subtracts the known count (-n_invalid(row) * bandwidth(x))
    from den. No kbias input, no per-row mask input.
  - den is transposed (PE) BEFORE the reciprocal so the divide runs on 96
    DVE lanes instead of 1.
  - out = relu((outT^T @ Wv) * recip) per query row; one DMA per band.

Slow path (any nonzero bias): the original f32r kernel (v1) below.
"""

import numpy as np
from contextlib import ExitStack

import ml_dtypes

import concourse.bass as bass
import concourse.bacc as bacc
import concourse.tile as tile
from concourse import mybir
from concourse.bass_utils import run_bass_kernel_spmd

DT = mybir.dt.float32
BF = mybir.dt.bfloat16
FR = mybir.dt.float32r
AF = mybir.ActivationFunctionType
BF_NP = ml_dtypes.bfloat16

# Problem constants (hardcoded per contract)
B, H, W, C, KD, OD = 2, 96, 96, 128, 64, 128
KS, PAD = 7, 3
NCORES = 8
ROWS = (B * H) // NCORES        # 24 query rows per core
KROWS = ROWS + 2 * PAD          # 30 k/v rows per core (with halo)
NQ = ROWS * W                   # 2304 query pixels per core
NK = KROWS * W                  # 2880 key pixels per core
BAND = 4                        # query rows per band
NBANDS = ROWS // BAND           # 6
BN = BAND * W                   # 384 band query columns
NKR = BAND + 2 * PAD            # 10 k-rows per band
SCALE = 1.0 / np.sqrt(KD)       # 1/8

# Per-band k-row geometry: k-row i serves query rows c in [C0[i], C0[i]+WID[i])
C0 = [max(0, i - 2 * PAD) for i in range(NKR)]
WID = [min(BAND - 1, i) - max(0, i - 2 * PAD) + 1 for i in range(NKR)]
# Packed score-tile layout: two phases of five k-rows each, 3 PSUM banks
# (1536 fp32 cols); offsets chosen so no block crosses a 512-col bank edge.
PH_I = [(0, 1, 2, 3, 4), (5, 6, 7, 8, 9)]
OFF = {0: 384, 1: 800, 2: 512, 3: 0, 4: 1024,
       5: 0, 6: 1024, 7: 512, 8: 800, 9: 384}
SPACK = 1536
SEG = ((0, 480), (512, 992), (1024, 1408))


def build_nc():
    nc = bacc.Bacc(None, target_bir_lowering=False)
    qt = nc.dram_tensor("qt", [C, NQ], BF, kind="ExternalInput")
    kt = nc.dram_tensor("kt", [C, NK], BF, kind="ExternalInput")
    v = nc.dram_tensor("v", [W, KROWS, OD], BF, kind="ExternalInput")
    gw = nc.dram_tensor("gw", [C, C + OD], BF, kind="ExternalInput")
    bandp = nc.dram_tensor("bandp", [W, SPACK], BF, kind="ExternalInput")
    wcorr = nc.dram_tensor("wcorr", [1, NBANDS * BN], BF, kind="ExternalInput")
    out = nc.dram_tensor("out", [ROWS, W, OD], DT, kind="ExternalOutput")

    with tile.TileContext(nc) as tc, ExitStack() as ctx:
        consts = ctx.enter_context(tc.tile_pool(name="consts", bufs=1))
        slabs = ctx.enter_context(tc.tile_pool(name="slabs", bufs=1))
        e_pool = ctx.enter_context(tc.tile_pool(name="e_pool", bufs=2))
        sm_pool = ctx.enter_context(tc.tile_pool(name="sm_pool", bufs=2))
        ot_pool = ctx.enter_context(tc.tile_pool(name="ot_pool", bufs=2))
        os_pool = ctx.enter_context(tc.tile_pool(name="os_pool", bufs=2))
        ps_s = ctx.enter_context(tc.tile_pool(name="ps_s", bufs=2, space="PSUM"))
        ps_o = ctx.enter_context(tc.tile_pool(name="ps_o", bufs=1, space="PSUM"))
        ps_d = ctx.enter_context(tc.tile_pool(name="ps_d", bufs=1, space="PSUM"))

        # ---- constants (gw first: kG needs it immediately) ----
        gw_s = consts.tile([C, C + OD], BF, tag="cgw")
        nc.sync.dma_start(out=gw_s[:], in_=gw[:])
        g_s = gw_s[:, :C]
        wv_s = gw_s[:, C : C + OD]
        ones1 = consts.tile([1, 1], DT, tag="cone1")
        nc.vector.memset(ones1[:], 1.0)
        oneb = consts.tile([1, 1], BF, tag="coneb")
        nc.vector.memset(oneb[:], 1.0)
        ones96 = consts.tile([W, 1], BF, tag="cones96")
        nc.vector.memset(ones96[:], 1.0)

        # ---- slabs; kt arrives in chunks so kG can start early ----
        kt_s = slabs.tile([C, NK], BF, tag="skt")
        for j0 in range(0, NK, 512):
            j1 = min(j0 + 512, NK)
            nc.sync.dma_start(out=kt_s[:, j0:j1], in_=kt[:, j0:j1])
        qt_s = slabs.tile([C, NQ], BF, tag="sqt")
        nc.sync.dma_start(out=qt_s[:], in_=qt[:])
        bandp_s = consts.tile([W, SPACK], BF, tag="cbp")
        nc.sync.dma_start(out=bandp_s[:], in_=bandp[:])
        v_s = slabs.tile([W, KROWS, OD], BF, tag="sv")
        nc.sync.dma_start(out=v_s[:], in_=v[:])
        wcorr_s = consts.tile([1, NBANDS * BN], BF, tag="cwc")
        nc.sync.dma_start(out=wcorr_s[:], in_=wcorr[:])

        # ---- kG = G^T-applied K slab: kG[:, p] = G @ k_pixel(p) ----
        kG_s = slabs.tile([C, NK], BF, tag="skG")

        def kg_chunk(j0):
            j1 = min(j0 + 512, NK)
            ps = ps_s.tile([C, 512], DT, tag="S")
            nc.tensor.matmul(
                out=ps[:, : j1 - j0], lhsT=g_s[:], rhs=kt_s[:, j0:j1],
                start=True, stop=True,
            )
            nc.scalar.copy(kG_s[:, j0:j1], ps[:, : j1 - j0])

        # ---- bands (software-pipelined: band P's tail fills band P+1's
        # exp window on PE, so the tensor engine never idles) ----
        st = [dict() for _ in range(NBANDS)]

        def tail_feed(P):
            # DVE feeders for band P's tail; emitted first so they run
            # before this band's masks occupy the vector queue.
            den_sb = sm_pool.tile([1, BN], DT, tag="densb")
            nc.vector.tensor_copy(den_sb[:], st[P]["den"][:])
            oT = ot_pool.tile([OD, BN], BF, tag="oT")
            nc.vector.tensor_copy(oT[:], st[P]["outT"][:])
            st[P]["den_sb"], st[P]["oT"] = den_sb, oT

        def tail_recip(P):
            # PE transposes of den (so the reciprocal runs on 96 lanes)
            denT = ps_o.tile([W, BAND], DT, tag="outT")
            den_sb = st[P]["den_sb"]
            for c in range(BAND):
                nc.tensor.transpose(
                    denT[:, c : c + 1], den_sb[:, c * W : (c + 1) * W], ones1[:]
                )
            recipT = sm_pool.tile([W, BAND], DT, tag="recipT")
            nc.vector.reciprocal(recipT[:], denT[:])
            st[P]["recipT"] = recipT

        def tail_out(P):
            # out-proj matmuls + relu*recip + store
            oT, recipT = st[P]["oT"], st[P]["recipT"]
            op = ps_d.tile([W, BAND * OD], DT, tag="den")
            ost = os_pool.tile([W, BAND * OD], DT, tag="ost")
            for c in range(BAND):
                nc.tensor.matmul(
                    out=op[:, c * OD : (c + 1) * OD],
                    lhsT=oT[:, c * W : (c + 1) * W],
                    rhs=wv_s[:],
                    start=True, stop=True,
                )
            for c in range(BAND):
                nc.vector.tensor_scalar(
                    ost[:, c * OD : (c + 1) * OD],
                    op[:, c * OD : (c + 1) * OD],
                    recipT[:, c : c + 1], 0.0,
                    mybir.AluOpType.mult, mybir.AluOpType.max,
                )
            h0p = P * BAND
            nc.sync.dma_start(
                out=out[h0p : h0p + BAND].rearrange("r x e -> x r e"),
                in_=ost[:].rearrange("x (r e) -> x r e", r=BAND),
            )

        for j0 in range(0, NK, 512):
            kg_chunk(j0)
        for band in range(NBANDS):
            h0 = band * BAND
            P = band - 1
            if P >= 0:
                tail_feed(P)
            Eph = []
            for ph in range(2):
                S = ps_s.tile([W, SPACK], DT, tag="S")
                for p0, p1 in ((480, 512), (992, 1024), (1408, 1536)):
                    nc.vector.memset(S[:, p0:p1], 0.0)
                for i in PH_I[ph]:
                    r, o, w = h0 + i, OFF[i], WID[i]
                    jq = slice((h0 + C0[i]) * W, (h0 + C0[i] + w) * W)
                    nc.tensor.matmul(
                        out=S[:, o : o + w * W],
                        lhsT=kG_s[:, r * W : (r + 1) * W],
                        rhs=qt_s[:, jq],
                        start=True, stop=True,
                    )
                E = e_pool.tile([W, SPACK], BF, tag="E")
                nc.scalar.activation(E[:], S[:], AF.Exp, bias=0.0, scale=SCALE)
                nc.vector.tensor_mul(E[:], E[:], bandp_s[:])
                Eph.append(E)
                if ph == 0 and P >= 0:
                    tail_recip(P)
            if P >= 0:
                tail_out(P)
            # accumulation: the full-width i=3 block is issued first with
            # start=True so every later sub-range write is pure accumulation
            outT = ps_o.tile([OD, BN], DT, tag="outT")
            den = ps_d.tile([1, BN], DT, tag="den")
            st[band]["outT"], st[band]["den"] = outT, den
            for ph in range(2):
                E = Eph[ph]
                order = (3, 0, 1, 2, 4) if ph == 0 else PH_I[1]
                for i in order:
                    r, o, w = h0 + i, OFF[i], WID[i]
                    js = slice(C0[i] * W, (C0[i] + w) * W)
                    nc.tensor.matmul(
                        out=outT[:, js],
                        lhsT=v_s[:, r, :],
                        rhs=E[:, o : o + w * W],
                        start=(ph == 0 and i == 3), stop=(i == NKR - 1),
                    )
                for i in order:
                    r, o, w = h0 + i, OFF[i], WID[i]
                    js = slice(C0[i] * W, (C0[i] + w) * W)
                    nc.tensor.matmul(
                        out=den[:, js],
                        lhsT=ones96[:],
                        rhs=E[:, o : o + w * W],
                        start=(ph == 0 and i == 3), stop=False,
                    )
            # den -= n_invalid(row) * bandwidth(x)  (zero for interior bands)
            nc.tensor.matmul(
                out=den[:], lhsT=oneb[:],
                rhs=wcorr_s[:, band * BN : (band + 1) * BN],
                start=False, stop=True,
            )

        # final band's tail
        Pl = NBANDS - 1
        tail_feed(Pl)
        tail_recip(Pl)
        tail_out(Pl)

    nc.compile()
    return nc


def _bf(x):
    return np.ascontiguousarray(np.asarray(x, np.float32)).astype(BF_NP)


def make_in_maps(Q, K, V, Wq, bq, Wk, bk, Wv, bv):
    Q = np.asarray(Q, np.float32)
    K = np.asarray(K, np.float32)
    V = np.asarray(V, np.float32)
    G = np.asarray(Wq, np.float32) @ np.asarray(Wk, np.float32).T  # [C, C]
    gw = np.concatenate([G.T, np.asarray(Wv, np.float32)], axis=1)  # [C, C+OD]
    gwb = _bf(gw)

    # band mask constant, packed-layout [96, 1536]
    idx = np.arange(W)
    band96 = (np.abs(idx[:, None] - idx[None, :]) <= PAD).astype(np.float32)
    bandp = np.zeros((W, SPACK), np.float32)
    for i in PH_I[0]:
        o, w = OFF[i], WID[i]
        bandp[:, o : o + w * W] = np.tile(band96, (1, w))
    bandp = _bf(bandp)

    bw = (np.minimum(idx + PAD, W - 1) - np.maximum(idx - PAD, 0) + 1).astype(
        np.float32
    )  # valid kx count per x

    in_maps = []
    for core in range(NCORES):
        b = core // (H // ROWS)
        h_start = (core % (H // ROWS)) * ROWS

        qs = Q[b, h_start : h_start + ROWS]              # [24,96,128]
        qtc = _bf(qs.reshape(NQ, C).T)                   # [128,2304]

        kpad = np.zeros((KROWS, W, C), np.float32)
        vpad = np.zeros((KROWS, W, C), np.float32)
        for j in range(KROWS):
            gr = h_start - PAD + j
            if 0 <= gr < H:
                kpad[j] = K[b, gr]
                vpad[j] = V[b, gr]
        ktc = _bf(kpad.reshape(NK, C).T)                 # [128,2880]
        vtc = _bf(vpad.transpose(1, 0, 2))               # [96,30,128]

        wcorr = np.zeros((1, NBANDS * BN), np.float32)
        for band in range(NBANDS):
            for c in range(BAND):
                gr = h_start + band * BAND + c
                n_inv = sum(
                    1 for dy in range(-PAD, PAD + 1) if not (0 <= gr + dy < H)
                )
                if n_inv:
                    wcorr[0, band * BN + c * W : band * BN + (c + 1) * W] = -n_inv * bw
        in_maps.append(
            {
                "qt": qtc,
                "kt": ktc,
                "v": vtc,
                "gw": gwb,
                "bandp": bandp,
                "wcorr": _bf(wcorr),
            }
        )
    return in_maps


def gather(results):
    full = np.empty((B, H, W, OD), np.float32)
    for core in range(NCORES):
        b = core // (H // ROWS)
        h_start = (core % (H // ROWS)) * ROWS
        full[b, h_start : h_start + ROWS] = results[core]["out"]
    return full


_NC_CACHE = {}


def get_nc(path="v2"):
    if path not in _NC_CACHE:
        _NC_CACHE[path] = build_nc() if path == "v2" else build_nc_v1(
            with_bv=(path == "v1bv")
        )
    return _NC_CACHE[path]


def kernel(Q, K, V, Wq, bq, Wk, bk, Wv, bv):
    if np.any(np.asarray(bq)) or np.any(np.asarray(bk)):
        nc = get_nc("v1bv" if np.any(np.asarray(bv)) else "v1")
        in_maps = make_in_maps_v1(Q, K, V, Wq, bq, Wk, bk, Wv, bv)
    elif np.any(np.asarray(bv)):
        nc = get_nc("v1bv")
        in_maps = make_in_maps_v1(Q, K, V, Wq, bq, Wk, bk, Wv, bv)
    else:
        nc = get_nc("v2")
        in_maps = make_in_maps(Q, K, V, Wq, bq, Wk, bk, Wv, bv)
    res = run_bass_kernel_spmd(nc, in_maps, list(range(NCORES)))
    return gather(res.results)


# ======================================================================
# v1 fallback (original f32r kernel) — used only when a bias is nonzero.
# ======================================================================

WVN = 2 * OD
NEG = -30000.0


def build_nc_v1(with_bv=False):
    MDT = FR
    nc = bacc.Bacc(None, target_bir_lowering=False)
    qt = nc.dram_tensor("qt", [C, NQ], MDT, kind="ExternalInput")
    kt = nc.dram_tensor("kt", [C, NK], MDT, kind="ExternalInput")
    v = nc.dram_tensor("v", [W, KROWS, C], MDT, kind="ExternalInput")
    wq = nc.dram_tensor("wq", [C, KD], MDT, kind="ExternalInput")
    wk = nc.dram_tensor("wk", [C, KD], MDT, kind="ExternalInput")
    wv = nc.dram_tensor("wv", [C, WVN], MDT, kind="ExternalInput")
    bq = nc.dram_tensor("bq", [KD, 1], DT, kind="ExternalInput")
    bk = nc.dram_tensor("bk", [KD, 1], DT, kind="ExternalInput")
    bv = nc.dram_tensor("bv", [1, WVN], MDT, kind="ExternalInput")
    kbias = nc.dram_tensor("kbias", [W, KROWS], DT, kind="ExternalInput")
    ones_in = nc.dram_tensor("ones", [W, 1], MDT, kind="ExternalInput")
    b4 = nc.dram_tensor("b4", [W, NKR * BN], DT, kind="ExternalInput")
    out = nc.dram_tensor("out", [ROWS, W, OD], DT, kind="ExternalOutput")

    with tile.TileContext(nc) as tc, ExitStack() as ctx:
        consts = ctx.enter_context(tc.tile_pool(name="consts", bufs=1))
        slabs = ctx.enter_context(tc.tile_pool(name="slabs", bufs=1))
        e_pool = ctx.enter_context(tc.tile_pool(name="e_pool", bufs=3))
        o_pool = ctx.enter_context(tc.tile_pool(name="o_pool", bufs=2))
        r_pool = ctx.enter_context(tc.tile_pool(name="r_pool", bufs=2))
        rs_pool = ctx.enter_context(tc.tile_pool(name="rs_pool", bufs=8))
        outs = ctx.enter_context(tc.tile_pool(name="outs", bufs=3))
        ps_a = ctx.enter_context(tc.tile_pool(name="ps_a", bufs=3, space="PSUM"))
        ps_b = ctx.enter_context(tc.tile_pool(name="ps_b", bufs=2, space="PSUM"))
        ps_c = ctx.enter_context(tc.tile_pool(name="ps_c", bufs=2, space="PSUM"))

        wq_s = consts.tile([C, KD], MDT, tag="cw")
        nc.sync.dma_start(out=wq_s[:], in_=wq[:])
        wk_s = consts.tile([C, KD], MDT, tag="cw2")
        nc.sync.dma_start(out=wk_s[:], in_=wk[:])
        wv_s = consts.tile([C, WVN], MDT, tag="cw3")
        nc.sync.dma_start(out=wv_s[:], in_=wv[:])
        bq_s = consts.tile([KD, 1], DT, tag="cb")
        nc.sync.dma_start(out=bq_s[:], in_=bq[:])
        bk_s = consts.tile([KD, 1], DT, tag="cb2")
        nc.sync.dma_start(out=bk_s[:], in_=bk[:])
        kbias_s = consts.tile([W, KROWS], DT, tag="ckb")
        nc.sync.dma_start(out=kbias_s[:], in_=kbias[:])
        b4_s = consts.tile([W, NKR * BN], DT, tag="cb4")
        nc.sync.dma_start(out=b4_s[:], in_=b4[:])
        ones96 = consts.tile([W, 1], MDT, tag="cones")
        nc.sync.dma_start(out=ones96[:], in_=ones_in[:])
        ones1 = consts.tile([1, 1], DT, tag="cone1")
        nc.vector.memset(ones1[:], 1.0)
        if with_bv:
            bv_s = consts.tile([1, WVN], MDT, tag="cbv")
            nc.sync.dma_start(out=bv_s[:], in_=bv[:])

        qt_s = slabs.tile([C, NQ], MDT, tag="sqt")
        nc.sync.dma_start(out=qt_s[:], in_=qt[:])
        kt_s = slabs.tile([C, NK], MDT, tag="skt")
        nc.sync.dma_start(out=kt_s[:], in_=kt[:])
        v_s = slabs.tile([W, KROWS, C], MDT, tag="sv")
        nc.sync.dma_start(out=v_s[:], in_=v[:])

        qT_s = slabs.tile([KD, NQ], MDT, tag="sqT")
        kT_s = slabs.tile([KD, NK], MDT, tag="skT")
        for dst, src, wmat, bvec, n in (
            (qT_s, qt_s, wq_s, bq_s, NQ),
            (kT_s, kt_s, wk_s, bk_s, NK),
        ):
            for j0 in range(0, n, 512):
                j1 = min(j0 + 512, n)
                ps = ps_a.tile([KD, 512], DT, tag="w")
                nc.tensor.matmul(
                    out=ps[:, : j1 - j0], lhsT=wmat[:], rhs=src[:, j0:j1],
                    start=True, stop=True,
                )
                nc.scalar.activation(
                    dst[:, j0:j1], ps[:, : j1 - j0], AF.Identity,
                    bias=bvec[:], scale=1.0,
                )

        for band in range(NBANDS):
            h0 = band * BAND
            jq = slice(h0 * W, (h0 + BAND) * W)
            outT = ps_b.tile([OD, BN], DT, tag="outT")
            den = ps_c.tile([1, BN], DT, tag="den")
            for i in range(NKR):
                r = h0 + i
                S = ps_a.tile([W, BN], DT, tag="w")
                nc.tensor.matmul(
                    out=S[:], lhsT=kT_s[:, r * W : (r + 1) * W],
                    rhs=qT_s[:, jq], start=True, stop=True,
                )
                E = e_pool.tile([W, BN], MDT, tag="E")
                nc.scalar.activation(
                    E[:], S[:], AF.Exp, bias=kbias_s[:, r : r + 1], scale=SCALE
                )
                nc.vector.tensor_mul(E[:], E[:], b4_s[:, i * BN : (i + 1) * BN])
                nc.tensor.matmul(
                    out=outT[:], lhsT=v_s[:, r, :], rhs=E[:],
                    start=(i == 0), stop=(i == NKR - 1),
                )
                nc.tensor.matmul(
                    out=den[:], lhsT=ones96[:], rhs=E[:],
                    start=(i == 0), stop=(i == NKR - 1),
                )

            recip = r_pool.tile([1, BN], DT, tag="recip")
            nc.vector.reciprocal(recip[:], den[:])
            oT = o_pool.tile([OD, BN], MDT, tag="oT")
            nc.vector.tensor_copy(oT[:], outT[:])
            if with_bv:
                den_sb = r_pool.tile([1, BN], MDT, tag="densb")
                nc.vector.tensor_copy(den_sb[:], den[:])
            for c in range(BAND):
                cs = slice(c * W, (c + 1) * W)
                rT = ps_a.tile([W, 1], DT, tag="w")
                nc.tensor.transpose(rT[:], recip[:, cs], ones1[:])
                rS = rs_pool.tile([W, 1], DT, tag="rS")
                nc.vector.tensor_copy(rS[:], rT[:])
                op = ps_a.tile([W, WVN], DT, tag="w")
                nc.tensor.matmul(
                    out=op[:], lhsT=oT[:, cs], rhs=wv_s[:],
                    start=True, stop=not with_bv,
                )
                if with_bv:
                    nc.tensor.matmul(
                        out=op[:], lhsT=den_sb[:, cs], rhs=bv_s[:],
                        start=False, stop=True,
                    )
                ost = outs.tile([W, OD], DT, tag="ost")
                nc.scalar.activation(ost[:], op[:, :OD], AF.Relu, bias=0.0, scale=rS[:])
                nc.sync.dma_start(out=out[h0 + c], in_=ost[:])

    nc.compile()
    return nc


def round_f32r(x):
    b = np.ascontiguousarray(x, np.float32).view(np.uint32)
    tie = (b >> 12) & 1
    b = (b + 0x7FF + tie) & np.uint32(0xFFFFF000)
    return b.view(np.float32)


def make_in_maps_v1(Q, K, V, Wq, bq, Wk, bk, Wv, bv):
    rnd = round_f32r
    Q = np.asarray(Q, np.float32)
    K = np.asarray(K, np.float32)
    V = np.asarray(V, np.float32)
    Wqr = rnd(np.asarray(Wq, np.float32))
    Wkr = rnd(np.asarray(Wk, np.float32))
    wvp = np.zeros((C, WVN), np.float32)
    wvp[:, :OD] = np.asarray(Wv, np.float32)
    wvp = rnd(wvp)
    bqv = np.ascontiguousarray(np.asarray(bq, np.float32).reshape(KD, 1))
    bkv = np.ascontiguousarray(np.asarray(bk, np.float32).reshape(KD, 1))
    bvp = np.zeros((1, WVN), np.float32)
    bvp[0, :OD] = np.asarray(bv, np.float32)
    bvp = rnd(bvp)

    idx = np.arange(W)
    b4 = (np.abs(idx[:, None] - idx[None, :]) <= PAD).astype(np.float32)
    b4i = np.zeros((W, NKR, BAND, W), np.float32)
    for i in range(NKR):
        for c in range(BAND):
            if i - 2 * PAD <= c <= i:
                b4i[:, i, c, :] = b4
    b4rep = np.ascontiguousarray(b4i.reshape(W, NKR * BAND * W))

    in_maps = []
    for core in range(NCORES):
        b = core // (H // ROWS)
        h_start = (core % (H // ROWS)) * ROWS
        qs = Q[b, h_start : h_start + ROWS]
        qtc = rnd(np.ascontiguousarray(qs.reshape(NQ, C).T))
        kpad = np.zeros((KROWS, W, C), np.float32)
        vpad = np.zeros((KROWS, W, C), np.float32)
        kb = np.full((KROWS,), NEG, np.float32)
        for j in range(KROWS):
            gr = h_start - PAD + j
            if 0 <= gr < H:
                kpad[j] = K[b, gr]
                vpad[j] = V[b, gr]
                kb[j] = 0.0
        ktc = rnd(np.ascontiguousarray(kpad.reshape(NK, C).T))
        vtc = rnd(np.ascontiguousarray(vpad.transpose(1, 0, 2)))
        kbias = np.ascontiguousarray(np.broadcast_to(kb[None, :], (W, KROWS)))
        in_maps.append(
            {
                "qt": qtc, "kt": ktc, "v": vtc,
                "wq": Wqr, "wk": Wkr, "wv": wvp,
                "bq": bqv, "bk": bkv, "bv": bvp,
                "kbias": kbias,
                "ones": np.ones((W, 1), np.float32),
                "b4": b4rep,
            }
        )
    return in_maps


# revision 11
# speedup vs baseline: 1.2445x; 1.0202x over previous
"""Trainium2 Bass kernel for BaseAttentionConvolution (7x7 neighborhood attention).

Computation (reference, fp32):
    q = Q @ Wq + bq                     # [B,H,W,64]
    k = K @ Wk + bk                     # [B,H,W,64]
    S[p, (dy,dx)] = q[p] . k[p+(dy,dx)]         (7x7 window, -inf outside image)
    P = softmax(S / 8)
    O[p] = sum_j P[p,j] * V[p+j]        # [B,H,W,128]
    out = relu(O @ Wv + bv)             # [B,H,W,128]

Sharding: B*H = 192 rows split into 8 bands of 24 rows (one per core).

Fast path (bq = bk = bv = 0, the shipped configuration), bf16 matmuls:
  - Host fuses G = Wq @ Wk^T so S = x_q^T G x_k needs no q/k projections:
    kG[128, 2880] = G^T-matmul over the raw K slab (on PE), and the raw Q
    slab is the moving operand of the score matmuls directly.
  - Bands of 4 query rows; for each of the 10 k-rows of a band only the
    valid query-row range is computed (widths 1,2,3,4,4,4,4,3,2,1 x 96),
    eliminating all redundant (q-row, k-row) pairs.
  - Per band the 10 score blocks are packed into two 3-PSUM-bank tiles
    (no matmul crosses a bank) so exp and band-masking run as one big
    ACT/DVE op per phase instead of ten small ones.
  - Image-edge handling: K/V halo rows are zeros, so a halo row
    contributes exp(0)*band = band to the softmax denominator; a rank-1
    matmul subtracts the known count (-n_invalid(row) * bandwidth(x))
    from den. No kbias input, no per-row mask input.
  - den is transposed (PE) BEFORE the reciprocal so the divide runs on 96
    DVE lanes instead of 1.
  - out = relu((outT^T @ Wv) * recip) per query row; one DMA per band.

Slow path (any nonzero bias): the original f32r kernel (v1) below.
"""

import numpy as np
from contextlib import ExitStack

import ml_dtypes

import concourse.bass as bass
import concourse.bacc as bacc
import concourse.tile as tile
from concourse import mybir
from concourse.bass_utils import run_bass_kernel_spmd

DT = mybir.dt.float32
BF = mybir.dt.bfloat16
FR = mybir.dt.float32r
AF = mybir.ActivationFunctionType
BF_NP = ml_dtypes.bfloat16

# Problem constants (hardcoded per contract)
B, H, W, C, KD, OD = 2, 96, 96, 128, 64, 128
KS, PAD = 7, 3
NCORES = 8
ROWS = (B * H) // NCORES        # 24 query rows per core
KROWS = ROWS + 2 * PAD          # 30 k/v rows per core (with halo)
NQ = ROWS * W                   # 2304 query pixels per core
NK = KROWS * W                  # 2880 key pixels per core
BAND = 4                        # query rows per band
NBANDS = ROWS // BAND           # 6
BN = BAND * W                   # 384 band query columns
NKR = BAND + 2 * PAD            # 10 k-rows per band
SCALE = 1.0 / np.sqrt(KD)       # 1/8

# Per-band k-row geometry: k-row i serves query rows c in [C0[i], C0[i]+WID[i])
C0 = [max(0, i - 2 * PAD) for i in range(NKR)]
WID = [min(BAND - 1, i) - max(0, i - 2 * PAD) + 1 for i in range(NKR)]
# Packed score-tile layout: two phases of five k-rows each, 3 PSUM banks
# (1536 fp32 cols); offsets chosen so no block crosses a 512-col bank edge.
PH_I = [(0, 1, 2, 3, 4), (5, 6, 7, 8, 9)]
OFF = {0: 384, 1: 800, 2: 512, 3: 0, 4: 1024,
       5: 0, 6: 1024, 7: 512, 8: 800, 9: 384}
SPACK = 1536
SEG = ((0, 480), (512, 992), (1024, 1408))


def build_nc():
    nc = bacc.Bacc(None, target_bir_lowering=False)
    qt = nc.dram_tensor("qt", [C, NQ], BF, kind="ExternalInput")
    kt = nc.dram_tensor("kt", [C, NK], BF, kind="ExternalInput")
    v = nc.dram_tensor("v", [W, KROWS, OD], BF, kind="ExternalInput")
    gw = nc.dram_tensor("gw", [C, C + OD], BF, kind="ExternalInput")
    bandp = nc.dram_tensor("bandp", [W, SPACK], BF, kind="ExternalInput")
    wcorr = nc.dram_tensor("wcorr", [1, NBANDS * BN], BF, kind="ExternalInput")
    out = nc.dram_tensor("out", [ROWS, W, OD], DT, kind="ExternalOutput")

    with tile.TileContext(nc) as tc, ExitStack() as ctx:
        consts = ctx.enter_context(tc.tile_pool(name="consts", bufs=1))
        slabs = ctx.enter_context(tc.tile_pool(name="slabs", bufs=1))
        e_pool = ctx.enter_context(tc.tile_pool(name="e_pool", bufs=2))
        sm_pool = ctx.enter_context(tc.tile_pool(name="sm_pool", bufs=2))
        ot_pool = ctx.enter_context(tc.tile_pool(name="ot_pool", bufs=2))
        os_pool = ctx.enter_context(tc.tile_pool(name="os_pool", bufs=2))
        ps_sa = ctx.enter_context(tc.tile_pool(name="ps_sa", bufs=1, space="PSUM"))
        ps_sb = ctx.enter_context(tc.tile_pool(name="ps_sb", bufs=1, space="PSUM"))
        ps_o = ctx.enter_context(tc.tile_pool(name="ps_o", bufs=1, space="PSUM"))
        ps_d = ctx.enter_context(tc.tile_pool(name="ps_d", bufs=1, space="PSUM"))

        # ---- constants (gw first: kG needs it immediately) ----
        gw_s = consts.tile([C, C + OD], BF, tag="cgw")
        nc.sync.dma_start(out=gw_s[:], in_=gw[:])
        g_s = gw_s[:, :C]
        wv_s = gw_s[:, C : C + OD]
        ones1 = consts.tile([1, 1], DT, tag="cone1")
        nc.vector.memset(ones1[:], 1.0)
        oneb = consts.tile([1, 1], BF, tag="coneb")
        nc.vector.memset(oneb[:], 1.0)
        ones96 = consts.tile([W, 1], BF, tag="cones96")
        nc.vector.memset(ones96[:], 1.0)

        # ---- slabs; kt arrives in chunks so kG can start early ----
        kt_s = slabs.tile([C, NK], BF, tag="skt")
        for j0 in range(0, NK, 512):
            j1 = min(j0 + 512, NK)
            nc.sync.dma_start(out=kt_s[:, j0:j1], in_=kt[:, j0:j1])
        qt_s = slabs.tile([C, NQ], BF, tag="sqt")
        nc.sync.dma_start(out=qt_s[:], in_=qt[:])
        bandp_s = consts.tile([W, SPACK], BF, tag="cbp")
        nc.sync.dma_start(out=bandp_s[:], in_=bandp[:])
        v_s = slabs.tile([W, KROWS, OD], BF, tag="sv")
        nc.sync.dma_start(out=v_s[:], in_=v[:])
        wcorr_s = consts.tile([1, NBANDS * BN], BF, tag="cwc")
        nc.sync.dma_start(out=wcorr_s[:], in_=wcorr[:])

        # ---- kG = G^T-applied K slab: kG[:, p] = G @ k_pixel(p) ----
        kG_s = slabs.tile([C, NK], BF, tag="skG")

        def kg_chunk(j0):
            j1 = min(j0 + 512, NK)
            ps = (ps_sa if (j0 // 512) % 2 == 0 else ps_sb).tile(
                [C, SPACK], DT, tag="S")
            nc.tensor.matmul(
                out=ps[:, : j1 - j0], lhsT=g_s[:], rhs=kt_s[:, j0:j1],
                start=True, stop=True,
            )
            nc.scalar.copy(kG_s[:, j0:j1], ps[:, : j1 - j0])

        # ---- bands (software-pipelined: band P's tail fills band P+1's
        # exp window on PE, so the tensor engine never idles) ----
        st = [dict() for _ in range(NBANDS)]

        def tail_feed(P):
            # DVE feeders for band P's tail; emitted first so they run
            # before this band's masks occupy the vector queue.
            den_sb = sm_pool.tile([1, BN], DT, tag="densb")
            nc.vector.tensor_copy(den_sb[:], st[P]["den"][:])
            oT = ot_pool.tile([OD, BN], BF, tag="oT")
            nc.vector.tensor_copy(oT[:], st[P]["outT"][:])
            st[P]["den_sb"], st[P]["oT"] = den_sb, oT

        def tail_recip(P):
            # PE transposes of den (so the reciprocal runs on 96 lanes)
            denT = ps_o.tile([W, BAND], DT, tag="outT")
            den_sb = st[P]["den_sb"]
            for c in range(BAND):
                nc.tensor.transpose(
                    denT[:, c : c + 1], den_sb[:, c * W : (c + 1) * W], ones1[:]
                )
            recipT = sm_pool.tile([W, BAND], DT, tag="recipT")
            nc.vector.reciprocal(recipT[:], denT[:])
            st[P]["recipT"] = recipT

        def tail_out(P):
            # out-proj matmuls + relu*recip + store
            oT, recipT = st[P]["oT"], st[P]["recipT"]
            op = ps_d.tile([W, BAND * OD], DT, tag="den")
            ost = os_pool.tile([W, BAND * OD], DT, tag="ost")
            for c in range(BAND):
                nc.tensor.matmul(
                    out=op[:, c * OD : (c + 1) * OD],
                    lhsT=oT[:, c * W : (c + 1) * W],
                    rhs=wv_s[:],
                    start=True, stop=True,
                )
            for c in range(BAND):
                nc.vector.tensor_scalar(
                    ost[:, c * OD : (c + 1) * OD],
                    op[:, c * OD : (c + 1) * OD],
                    recipT[:, c : c + 1], 0.0,
                    mybir.AluOpType.mult, mybir.AluOpType.max,
                )
            h0p = P * BAND
            nc.sync.dma_start(
                out=out[h0p : h0p + BAND].rearrange("r x e -> x r e"),
                in_=ost[:].rearrange("x (r e) -> x r e", r=BAND),
            )

        for j0 in range(0, NK, 512):
            kg_chunk(j0)
        for band in range(NBANDS):
            h0 = band * BAND
            P = band - 1
            if P >= 0:
                tail_feed(P)
            Eph = []
            for ph in range(2):
                S = (ps_sa if ph == 0 else ps_sb).tile([W, SPACK], DT, tag="S")
                for p0, p1 in ((480, 512), (992, 1024), (1408, 1536)):
                    nc.vector.memset(S[:, p0:p1], 0.0)
                for i in PH_I[ph]:
                    r, o, w = h0 + i, OFF[i], WID[i]
                    jq = slice((h0 + C0[i]) * W, (h0 + C0[i] + w) * W)
                    nc.tensor.matmul(
                        out=S[:, o : o + w * W],
                        lhsT=kG_s[:, r * W : (r + 1) * W],
                        rhs=qt_s[:, jq],
                        start=True, stop=True,
                    )
                E = e_pool.tile([W, SPACK], BF, tag="E")
                nc.scalar.activation(E[:], S[:], AF.Exp, bias=0.0, scale=SCALE)
                nc.vector.tensor_mul(E[:], E[:], bandp_s[:])
                Eph.append(E)
                if ph == 0 and P >= 0:
                    tail_recip(P)
            if P >= 0:
                tail_out(P)
            # accumulation: the full-width i=3 block is issued first with
            # start=True so every later sub-range write is pure accumulation
            outT = ps_o.tile([OD, BN], DT, tag="outT")
            den = ps_d.tile([1, BN], DT, tag="den")
            st[band]["outT"], st[band]["den"] = outT, den
            for ph in range(2):
                E = Eph[ph]
                order = (3, 0, 1, 2, 4) if ph == 0 else PH_I[1]
                for i in order:
                    r, o, w = h0 + i, OFF[i], WID[i]
                    js = slice(C0[i] * W, (C0[i] + w) * W)
                    nc.tensor.matmul(
                        out=outT[:, js],
                        lhsT=v_s[:, r, :],
                        rhs=E[:, o : o + w * W],
                        start=(ph == 0 and i == 3), stop=(i == NKR - 1),
                    )
                for i in order:
                    r, o, w = h0 + i, OFF[i], WID[i]
                    js = slice(C0[i] * W, (C0[i] + w) * W)
                    nc.tensor.matmul(
                        out=den[:, js],
                        lhsT=ones96[:],
                        rhs=E[:, o : o + w * W],
                        start=(ph == 0 and i == 3), stop=False,
                    )
            # den -= n_invalid(row) * bandwidth(x)  (zero for interior bands)
            nc.tensor.matmul(
                out=den[:], lhsT=oneb[:],
                rhs=wcorr_s[:, band * BN : (band + 1) * BN],
                start=False, stop=True,
            )

        # final band's tail
        Pl = NBANDS - 1
        tail_feed(Pl)
        tail_recip(Pl)
        tail_out(Pl)

    nc.compile()
    return nc


def _bf(x):
    return np.ascontiguousarray(np.asarray(x, np.float32)).astype(BF_NP)


def make_in_maps(Q, K, V, Wq, bq, Wk, bk, Wv, bv):
    Q = np.asarray(Q, np.float32)
    K = np.asarray(K, np.float32)
    V = np.asarray(V, np.float32)
    G = np.asarray(Wq, np.float32) @ np.asarray(Wk, np.float32).T  # [C, C]
    gw = np.concatenate([G.T, np.asarray(Wv, np.float32)], axis=1)  # [C, C+OD]
    gwb = _bf(gw)

    # band mask constant, packed-layout [96, 1536]
    idx = np.arange(W)
    band96 = (np.abs(idx[:, None] - idx[None, :]) <= PAD).astype(np.float32)
    bandp = np.zeros((W, SPACK), np.float32)
    for i in PH_I[0]:
        o, w = OFF[i], WID[i]
        bandp[:, o : o + w * W] = np.tile(band96, (1, w))
    bandp = _bf(bandp)

    bw = (np.minimum(idx + PAD, W - 1) - np.maximum(idx - PAD, 0) + 1).astype(
        np.float32
    )  # valid kx count per x

    in_maps = []
    for core in range(NCORES):
        b = core // (H // ROWS)
        h_start = (core % (H // ROWS)) * ROWS

        qs = Q[b, h_start : h_start + ROWS]              # [24,96,128]
        qtc = _bf(qs.reshape(NQ, C).T)                   # [128,2304]

        kpad = np.zeros((KROWS, W, C), np.float32)
        vpad = np.zeros((KROWS, W, C), np.float32)
        for j in range(KROWS):
            gr = h_start - PAD + j
            if 0 <= gr < H:
                kpad[j] = K[b, gr]
                vpad[j] = V[b, gr]
        ktc = _bf(kpad.reshape(NK, C).T)                 # [128,2880]
        vtc = _bf(vpad.transpose(1, 0, 2))               # [96,30,128]

        wcorr = np.zeros((1, NBANDS * BN), np.float32)
        for band in range(NBANDS):
            for c in range(BAND):
                gr = h_start + band * BAND + c
                n_inv = sum(
                    1 for dy in range(-PAD, PAD + 1) if not (0 <= gr + dy < H)
                )
                if n_inv:
                    wcorr[0, band * BN + c * W : band * BN + (c + 1) * W] = -n_inv * bw
        in_maps.append(
            {
                "qt": qtc,
                "kt": ktc,
                "v": vtc,
                "gw": gwb,
                "bandp": bandp,
                "wcorr": _bf(wcorr),
            }
        )
    return in_maps


def gather(results):
    full = np.empty((B, H, W, OD), np.float32)
    for core in range(NCORES):
        b = core // (H // ROWS)
        h_start = (core % (H // ROWS)) * ROWS
        full[b, h_start : h_start + ROWS] = results[core]["out"]
    return full


_NC_CACHE = {}


def get_nc(path="v2"):
    if path not in _NC_CACHE:
        _NC_CACHE[path] = build_nc() if path == "v2" else build_nc_v1(
            with_bv=(path == "v1bv")
        )
    return _NC_CACHE[path]


def kernel(Q, K, V, Wq, bq, Wk, bk, Wv, bv):
    if np.any(np.asarray(bq)) or np.any(np.asarray(bk)):
        nc = get_nc("v1bv" if np.any(np.asarray(bv)) else "v1")
        in_maps = make_in_maps_v1(Q, K, V, Wq, bq, Wk, bk, Wv, bv)
    elif np.any(np.asarray(bv)):
        nc = get_nc("v1bv")
        in_maps = make_in_maps_v1(Q, K, V, Wq, bq, Wk, bk, Wv, bv)
    else:
        nc = get_nc("v2")
        in_maps = make_in_maps(Q, K, V, Wq, bq, Wk, bk, Wv, bv)
    res = run_bass_kernel_spmd(nc, in_maps, list(range(NCORES)))
    return gather(res.results)


# ======================================================================
# v1 fallback (original f32r kernel) — used only when a bias is nonzero.
# ======================================================================

WVN = 2 * OD
NEG = -30000.0


def build_nc_v1(with_bv=False):
    MDT = FR
    nc = bacc.Bacc(None, target_bir_lowering=False)
    qt = nc.dram_tensor("qt", [C, NQ], MDT, kind="ExternalInput")
    kt = nc.dram_tensor("kt", [C, NK], MDT, kind="ExternalInput")
    v = nc.dram_tensor("v", [W, KROWS, C], MDT, kind="ExternalInput")
    wq = nc.dram_tensor("wq", [C, KD], MDT, kind="ExternalInput")
    wk = nc.dram_tensor("wk", [C, KD], MDT, kind="ExternalInput")
    wv = nc.dram_tensor("wv", [C, WVN], MDT, kind="ExternalInput")
    bq = nc.dram_tensor("bq", [KD, 1], DT, kind="ExternalInput")
    bk = nc.dram_tensor("bk", [KD, 1], DT, kind="ExternalInput")
    bv = nc.dram_tensor("bv", [1, WVN], MDT, kind="ExternalInput")
    kbias = nc.dram_tensor("kbias", [W, KROWS], DT, kind="ExternalInput")
    ones_in = nc.dram_tensor("ones", [W, 1], MDT, kind="ExternalInput")
    b4 = nc.dram_tensor("b4", [W, NKR * BN], DT, kind="ExternalInput")
    out = nc.dram_tensor("out", [ROWS, W, OD], DT, kind="ExternalOutput")

    with tile.TileContext(nc) as tc, ExitStack() as ctx:
        consts = ctx.enter_context(tc.tile_pool(name="consts", bufs=1))
        slabs = ctx.enter_context(tc.tile_pool(name="slabs", bufs=1))
        e_pool = ctx.enter_context(tc.tile_pool(name="e_pool", bufs=3))
        o_pool = ctx.enter_context(tc.tile_pool(name="o_pool", bufs=2))
        r_pool = ctx.enter_context(tc.tile_pool(name="r_pool", bufs=2))
        rs_pool = ctx.enter_context(tc.tile_pool(name="rs_pool", bufs=8))
        outs = ctx.enter_context(tc.tile_pool(name="outs", bufs=3))
        ps_a = ctx.enter_context(tc.tile_pool(name="ps_a", bufs=3, space="PSUM"))
        ps_b = ctx.enter_context(tc.tile_pool(name="ps_b", bufs=2, space="PSUM"))
        ps_c = ctx.enter_context(tc.tile_pool(name="ps_c", bufs=2, space="PSUM"))

        wq_s = consts.tile([C, KD], MDT, tag="cw")
        nc.sync.dma_start(out=wq_s[:], in_=wq[:])
        wk_s = consts.tile([C, KD], MDT, tag="cw2")
        nc.sync.dma_start(out=wk_s[:], in_=wk[:])
        wv_s = consts.tile([C, WVN], MDT, tag="cw3")
        nc.sync.dma_start(out=wv_s[:], in_=wv[:])
        bq_s = consts.tile([KD, 1], DT, tag="cb")
        nc.sync.dma_start(out=bq_s[:], in_=bq[:])
        bk_s = consts.tile([KD, 1], DT, tag="cb2")
        nc.sync.dma_start(out=bk_s[:], in_=bk[:])
        kbias_s = consts.tile([W, KROWS], DT, tag="ckb")
        nc.sync.dma_start(out=kbias_s[:], in_=kbias[:])
        b4_s = consts.tile([W, NKR * BN], DT, tag="cb4")
        nc.sync.dma_start(out=b4_s[:], in_=b4[:])
        ones96 = consts.tile([W, 1], MDT, tag="cones")
        nc.sync.dma_start(out=ones96[:], in_=ones_in[:])
        ones1 = consts.tile([1, 1], DT, tag="cone1")
        nc.vector.memset(ones1[:], 1.0)
        if with_bv:
            bv_s = consts.tile([1, WVN], MDT, tag="cbv")
            nc.sync.dma_start(out=bv_s[:], in_=bv[:])

        qt_s = slabs.tile([C, NQ], MDT, tag="sqt")
        nc.sync.dma_start(out=qt_s[:], in_=qt[:])
        kt_s = slabs.tile([C, NK], MDT, tag="skt")
        nc.sync.dma_start(out=kt_s[:], in_=kt[:])
        v_s = slabs.tile([W, KROWS, C], MDT, tag="sv")
        nc.sync.dma_start(out=v_s[:], in_=v[:])

        qT_s = slabs.tile([KD, NQ], MDT, tag="sqT")
        kT_s = slabs.tile([KD, NK], MDT, tag="skT")
        for dst, src, wmat, bvec, n in (
            (qT_s, qt_s, wq_s, bq_s, NQ),
            (kT_s, kt_s, wk_s, bk_s, NK),
        ):
            for j0 in range(0, n, 512):
                j1 = min(j0 + 512, n)
                ps = ps_a.tile([KD, 512], DT, tag="w")
                nc.tensor.matmul(
                    out=ps[:, : j1 - j0], lhsT=wmat[:], rhs=src[:, j0:j1],
                    start=True, stop=True,
                )
                nc.scalar.activation(
                    dst[:, j0:j1], ps[:, : j1 - j0], AF.Identity,
                    bias=bvec[:], scale=1.0,
                )

        for band in range(NBANDS):
            h0 = band * BAND
            jq = slice(h0 * W, (h0 + BAND) * W)
            outT = ps_b.tile([OD, BN], DT, tag="outT")
            den = ps_c.tile([1, BN], DT, tag="den")
            for i in range(NKR):
                r = h0 + i
                S = ps_a.tile([W, BN], DT, tag="w")
                nc.tensor.matmul(
                    out=S[:], lhsT=kT_s[:, r * W : (r + 1) * W],
                    rhs=qT_s[:, jq], start=True, stop=True,
                )
                E = e_pool.tile([W, BN], MDT, tag="E")
                nc.scalar.activation(
                    E[:], S[:], AF.Exp, bias=kbias_s[:, r : r + 1], scale=SCALE
                )
                nc.vector.tensor_mul(E[:], E[:], b4_s[:, i * BN : (i + 1) * BN])
                nc.tensor.matmul(
                    out=outT[:], lhsT=v_s[:, r, :], rhs=E[:],
                    start=(i == 0), stop=(i == NKR - 1),
                )
                nc.tensor.matmul(
                    out=den[:], lhsT=ones96[:], rhs=E[:],
                    start=(i == 0), stop=(i == NKR - 1),
                )

            recip = r_pool.tile([1, BN], DT, tag="recip")
            nc.vector.reciprocal(recip[:], den[:])
            oT = o_pool.tile([OD, BN], MDT, tag="oT")
            nc.vector.tensor_copy(oT[:], outT[:])
            if with_bv:
                den_sb = r_pool.tile([1, BN], MDT, tag="densb")
                nc.vector.tensor_copy(den_sb[:], den[:])
            for c in range(BAND):
                cs = slice(c * W, (c + 1) * W)
                rT = ps_a.tile([W, 1], DT, tag="w")
                nc.tensor.transpose(rT[:], recip[:, cs], ones1[:])
                rS = rs_pool.tile([W, 1], DT, tag="rS")
                nc.vector.tensor_copy(rS[:], rT[:])
                op = ps_a.tile([W, WVN], DT, tag="w")
                nc.tensor.matmul(
                    out=op[:], lhsT=oT[:, cs], rhs=wv_s[:],
                    start=True, stop=not with_bv,
                )
                if with_bv:
                    nc.tensor.matmul(
                        out=op[:], lhsT=den_sb[:, cs], rhs=bv_s[:],
                        start=False, stop=True,
                    )
                ost = outs.tile([W, OD], DT, tag="ost")
                nc.scalar.activation(ost[:], op[:, :OD], AF.Relu, bias=0.0, scale=rS[:])
                nc.sync.dma_start(out=out[h0 + c], in_=ost[:])

    nc.compile()
    return nc


def round_f32r(x):
    b = np.ascontiguousarray(x, np.float32).view(np.uint32)
    tie = (b >> 12) & 1
    b = (b + 0x7FF + tie) & np.uint32(0xFFFFF000)
    return b.view(np.float32)


def make_in_maps_v1(Q, K, V, Wq, bq, Wk, bk, Wv, bv):
    rnd = round_f32r
    Q = np.asarray(Q, np.float32)
    K = np.asarray(K, np.float32)
    V = np.asarray(V, np.float32)
    Wqr = rnd(np.asarray(Wq, np.float32))
    Wkr = rnd(np.asarray(Wk, np.float32))
    wvp = np.zeros((C, WVN), np.float32)
    wvp[:, :OD] = np.asarray(Wv, np.float32)
    wvp = rnd(wvp)
    bqv = np.ascontiguousarray(np.asarray(bq, np.float32).reshape(KD, 1))
    bkv = np.ascontiguousarray(np.asarray(bk, np.float32).reshape(KD, 1))
    bvp = np.zeros((1, WVN), np.float32)
    bvp[0, :OD] = np.asarray(bv, np.float32)
    bvp = rnd(bvp)

    idx = np.arange(W)
    b4 = (np.abs(idx[:, None] - idx[None, :]) <= PAD).astype(np.float32)
    b4i = np.zeros((W, NKR, BAND, W), np.float32)
    for i in range(NKR):
        for c in range(BAND):
            if i - 2 * PAD <= c <= i:
                b4i[:, i, c, :] = b4
    b4rep = np.ascontiguousarray(b4i.reshape(W, NKR * BAND * W))

    in_maps = []
    for core in range(NCORES):
        b = core // (H // ROWS)
        h_start = (core % (H // ROWS)) * ROWS
        qs = Q[b, h_start : h_start + ROWS]
        qtc = rnd(np.ascontiguousarray(qs.reshape(NQ, C).T))
        kpad = np.zeros((KROWS, W, C), np.float32)
        vpad = np.zeros((KROWS, W, C), np.float32)
        kb = np.full((KROWS,), NEG, np.float32)
        for j in range(KROWS):
            gr = h_start - PAD + j
            if 0 <= gr < H:
                kpad[j] = K[b, gr]
                vpad[j] = V[b, gr]
                kb[j] = 0.0
        ktc = rnd(np.ascontiguousarray(kpad.reshape(NK, C).T))
        vtc = rnd(np.ascontiguousarray(vpad.transpose(1, 0, 2)))
        kbias = np.ascontiguousarray(np.broadcast_to(kb[None, :], (W, KROWS)))
        in_maps.append(
            {
                "qt": qtc, "kt": ktc, "v": vtc,
                "wq": Wqr, "wk": Wkr, "wv": wvp,
                "bq": bqv, "bk": bkv, "bv": bvp,
                "kbias": kbias,
                "ones": np.ones((W, 1), np.float32),
                "b4": b4rep,
            }
        )
    return in_maps


# revision 12
# speedup vs baseline: 1.2593x; 1.0119x over previous
"""Trainium2 Bass kernel for BaseAttentionConvolution (7x7 neighborhood attention).

Computation (reference, fp32):
    q = Q @ Wq + bq                     # [B,H,W,64]
    k = K @ Wk + bk                     # [B,H,W,64]
    S[p, (dy,dx)] = q[p] . k[p+(dy,dx)]         (7x7 window, -inf outside image)
    P = softmax(S / 8)
    O[p] = sum_j P[p,j] * V[p+j]        # [B,H,W,128]
    out = relu(O @ Wv + bv)             # [B,H,W,128]

Sharding: B*H = 192 rows split into 8 bands of 24 rows (one per core).

Fast path (bq = bk = bv = 0, the shipped configuration), bf16 matmuls:
  - Host fuses G = Wq @ Wk^T so S = x_q^T G x_k needs no q/k projections:
    kG[128, 2880] = G^T-matmul over the raw K slab (on PE), and the raw Q
    slab is the moving operand of the score matmuls directly.
  - Bands of 4 query rows; for each of the 10 k-rows of a band only the
    valid query-row range is computed (widths 1,2,3,4,4,4,4,3,2,1 x 96),
    eliminating all redundant (q-row, k-row) pairs.
  - Per band the 10 score blocks are packed into two 3-PSUM-bank tiles
    (no matmul crosses a bank) so exp and band-masking run as one big
    ACT/DVE op per phase instead of ten small ones.
  - Image-edge handling: K/V halo rows are zeros, so a halo row
    contributes exp(0)*band = band to the softmax denominator; a rank-1
    matmul subtracts the known count (-n_invalid(row) * bandwidth(x))
    from den. No kbias input, no per-row mask input.
  - den is transposed (PE) BEFORE the reciprocal so the divide runs on 96
    DVE lanes instead of 1.
  - out = relu((outT^T @ Wv) * recip) per query row; one DMA per band.

Slow path (any nonzero bias): the original f32r kernel (v1) below.
"""

import numpy as np
from contextlib import ExitStack

import ml_dtypes

import concourse.bass as bass
import concourse.bacc as bacc
import concourse.tile as tile
from concourse import mybir
from concourse.bass_utils import run_bass_kernel_spmd

DT = mybir.dt.float32
BF = mybir.dt.bfloat16
FR = mybir.dt.float32r
AF = mybir.ActivationFunctionType
BF_NP = ml_dtypes.bfloat16

# Problem constants (hardcoded per contract)
B, H, W, C, KD, OD = 2, 96, 96, 128, 64, 128
KS, PAD = 7, 3
NCORES = 8
ROWS = (B * H) // NCORES        # 24 query rows per core
KROWS = ROWS + 2 * PAD          # 30 k/v rows per core (with halo)
NQ = ROWS * W                   # 2304 query pixels per core
NK = KROWS * W                  # 2880 key pixels per core
BAND = 4                        # query rows per band
NBANDS = ROWS // BAND           # 6
BN = BAND * W                   # 384 band query columns
NKR = BAND + 2 * PAD            # 10 k-rows per band
SCALE = 1.0 / np.sqrt(KD)       # 1/8

# Per-band k-row geometry: k-row i serves query rows c in [C0[i], C0[i]+WID[i])
C0 = [max(0, i - 2 * PAD) for i in range(NKR)]
WID = [min(BAND - 1, i) - max(0, i - 2 * PAD) + 1 for i in range(NKR)]
# Packed score-tile layout: two phases of five k-rows each, 3 PSUM banks
# (1536 fp32 cols); offsets chosen so no block crosses a 512-col bank edge.
PH_I = [(0, 1, 2, 3, 4), (5, 6, 7, 8, 9)]
OFF = {0: 384, 1: 800, 2: 512, 3: 0, 4: 1024,
       5: 0, 6: 1024, 7: 512, 8: 800, 9: 384}
SPACK = 1536
SEG = ((0, 480), (512, 992), (1024, 1408))


def build_nc():
    nc = bacc.Bacc(None, target_bir_lowering=False)
    qt = nc.dram_tensor("qt", [C, NQ], BF, kind="ExternalInput")
    kt = nc.dram_tensor("kt", [C, NK], BF, kind="ExternalInput")
    v = nc.dram_tensor("v", [W, KROWS, OD], BF, kind="ExternalInput")
    gw = nc.dram_tensor("gw", [C, C + OD], BF, kind="ExternalInput")
    bandp = nc.dram_tensor("bandp", [W, SPACK], BF, kind="ExternalInput")
    wcorr = nc.dram_tensor("wcorr", [1, NBANDS * BN], BF, kind="ExternalInput")
    out = nc.dram_tensor("out", [ROWS, W, OD], DT, kind="ExternalOutput")

    with tile.TileContext(nc) as tc, ExitStack() as ctx:
        consts = ctx.enter_context(tc.tile_pool(name="consts", bufs=1))
        slabs = ctx.enter_context(tc.tile_pool(name="slabs", bufs=1))
        e_pool = ctx.enter_context(tc.tile_pool(name="e_pool", bufs=2))
        sm_pool = ctx.enter_context(tc.tile_pool(name="sm_pool", bufs=2))
        ot_pool = ctx.enter_context(tc.tile_pool(name="ot_pool", bufs=2))
        os_pool = ctx.enter_context(tc.tile_pool(name="os_pool", bufs=2))
        ps_sa = ctx.enter_context(tc.tile_pool(name="ps_sa", bufs=1, space="PSUM"))
        ps_sb = ctx.enter_context(tc.tile_pool(name="ps_sb", bufs=1, space="PSUM"))
        ps_o = ctx.enter_context(tc.tile_pool(name="ps_o", bufs=1, space="PSUM"))
        ps_d = ctx.enter_context(tc.tile_pool(name="ps_d", bufs=1, space="PSUM"))

        # ---- constants (gw first: kG needs it immediately) ----
        gw_s = consts.tile([C, C + OD], BF, tag="cgw")
        nc.sync.dma_start(out=gw_s[:], in_=gw[:])
        g_s = gw_s[:, :C]
        wv_s = gw_s[:, C : C + OD]
        ones1 = consts.tile([1, 1], DT, tag="cone1")
        nc.vector.memset(ones1[:], 1.0)
        oneb = consts.tile([1, 1], BF, tag="coneb")
        nc.vector.memset(oneb[:], 1.0)
        ones96 = consts.tile([W, 1], BF, tag="cones96")
        nc.vector.memset(ones96[:], 1.0)

        # ---- slabs; kt arrives in chunks so kG can start early ----
        kt_s = slabs.tile([C, NK], BF, tag="skt")
        for j0 in range(0, NK, 512):
            j1 = min(j0 + 512, NK)
            eng = nc.sync if (j0 // 512) % 2 == 0 else nc.scalar
            eng.dma_start(out=kt_s[:, j0:j1], in_=kt[:, j0:j1])
        qt_s = slabs.tile([C, NQ], BF, tag="sqt")
        nc.sync.dma_start(out=qt_s[:], in_=qt[:])
        bandp_s = consts.tile([W, SPACK], BF, tag="cbp")
        nc.scalar.dma_start(out=bandp_s[:], in_=bandp[:])
        v_s = slabs.tile([W, KROWS, OD], BF, tag="sv")
        nc.scalar.dma_start(out=v_s[:], in_=v[:])
        wcorr_s = consts.tile([1, NBANDS * BN], BF, tag="cwc")
        nc.sync.dma_start(out=wcorr_s[:], in_=wcorr[:])

        # ---- kG = G^T-applied K slab: kG[:, p] = G @ k_pixel(p) ----
        kG_s = slabs.tile([C, NK], BF, tag="skG")

        def kg_chunk(j0):
            j1 = min(j0 + 512, NK)
            ps = (ps_sa if (j0 // 512) % 2 == 0 else ps_sb).tile(
                [C, SPACK], DT, tag="S")
            nc.tensor.matmul(
                out=ps[:, : j1 - j0], lhsT=g_s[:], rhs=kt_s[:, j0:j1],
                start=True, stop=True,
            )
            nc.scalar.copy(kG_s[:, j0:j1], ps[:, : j1 - j0])

        # ---- bands (software-pipelined: band P's tail fills band P+1's
        # exp window on PE, so the tensor engine never idles) ----
        st = [dict() for _ in range(NBANDS)]

        def tail_feed(P):
            # DVE feeders for band P's tail; emitted first so they run
            # before this band's masks occupy the vector queue.
            den_sb = sm_pool.tile([1, BN], DT, tag="densb")
            nc.vector.tensor_copy(den_sb[:], st[P]["den"][:])
            oT = ot_pool.tile([OD, BN], BF, tag="oT")
            nc.vector.tensor_copy(oT[:], st[P]["outT"][:])
            st[P]["den_sb"], st[P]["oT"] = den_sb, oT

        def tail_recip(P):
            # PE transposes of den (so the reciprocal runs on 96 lanes)
            denT = ps_o.tile([W, BAND], DT, tag="outT")
            den_sb = st[P]["den_sb"]
            for c in range(BAND):
                nc.tensor.transpose(
                    denT[:, c : c + 1], den_sb[:, c * W : (c + 1) * W], ones1[:]
                )
            recipT = sm_pool.tile([W, BAND], DT, tag="recipT")
            nc.vector.reciprocal(recipT[:], denT[:])
            st[P]["recipT"] = recipT

        def tail_out(P):
            # out-proj matmuls + relu*recip + store
            oT, recipT = st[P]["oT"], st[P]["recipT"]
            op = ps_d.tile([W, BAND * OD], DT, tag="den")
            ost = os_pool.tile([W, BAND * OD], DT, tag="ost")
            for c in range(BAND):
                nc.tensor.matmul(
                    out=op[:, c * OD : (c + 1) * OD],
                    lhsT=oT[:, c * W : (c + 1) * W],
                    rhs=wv_s[:],
                    start=True, stop=True,
                )
            for c in range(BAND):
                nc.scalar.activation(
                    ost[:, c * OD : (c + 1) * OD],
                    op[:, c * OD : (c + 1) * OD],
                    AF.Relu, bias=0.0, scale=recipT[:, c : c + 1],
                )
            h0p = P * BAND
            nc.sync.dma_start(
                out=out[h0p : h0p + BAND].rearrange("r x e -> x r e"),
                in_=ost[:].rearrange("x (r e) -> x r e", r=BAND),
            )

        for j0 in range(0, NK, 512):
            kg_chunk(j0)
        for band in range(NBANDS):
            h0 = band * BAND
            P = band - 1
            if P >= 0:
                tail_feed(P)
            Eph = []
            for ph in range(2):
                S = (ps_sa if ph == 0 else ps_sb).tile([W, SPACK], DT, tag="S")
                for p0, p1 in ((480, 512), (992, 1024), (1408, 1536)):
                    nc.vector.memset(S[:, p0:p1], 0.0)
                for i in PH_I[ph]:
                    r, o, w = h0 + i, OFF[i], WID[i]
                    jq = slice((h0 + C0[i]) * W, (h0 + C0[i] + w) * W)
                    nc.tensor.matmul(
                        out=S[:, o : o + w * W],
                        lhsT=kG_s[:, r * W : (r + 1) * W],
                        rhs=qt_s[:, jq],
                        start=True, stop=True,
                    )
                E = e_pool.tile([W, SPACK], BF, tag="E")
                nc.scalar.activation(E[:], S[:], AF.Exp, bias=0.0, scale=SCALE)
                nc.vector.tensor_mul(E[:, :480], E[:, :480], bandp_s[:, :480])
                nc.vector.tensor_mul(E[:, 480:], E[:, 480:], bandp_s[:, 480:])
                Eph.append(E)
                if ph == 0 and P >= 0:
                    tail_recip(P)
            if P >= 0:
                tail_out(P)
            # accumulation: the full-width i=3 block is issued first with
            # start=True so every later sub-range write is pure accumulation
            outT = ps_o.tile([OD, BN], DT, tag="outT")
            den = ps_d.tile([1, BN], DT, tag="den")
            st[band]["outT"], st[band]["den"] = outT, den
            for ph in range(2):
                E = Eph[ph]
                order = (3, 0, 1, 2, 4) if ph == 0 else PH_I[1]
                for i in order:
                    r, o, w = h0 + i, OFF[i], WID[i]
                    js = slice(C0[i] * W, (C0[i] + w) * W)
                    nc.tensor.matmul(
                        out=outT[:, js],
                        lhsT=v_s[:, r, :],
                        rhs=E[:, o : o + w * W],
                        start=(ph == 0 and i == 3), stop=(i == NKR - 1),
                    )
                for i in order:
                    r, o, w = h0 + i, OFF[i], WID[i]
                    js = slice(C0[i] * W, (C0[i] + w) * W)
                    nc.tensor.matmul(
                        out=den[:, js],
                        lhsT=ones96[:],
                        rhs=E[:, o : o + w * W],
                        start=(ph == 0 and i == 3), stop=False,
                    )
            # den -= n_invalid(row) * bandwidth(x)  (zero for interior bands)
            nc.tensor.matmul(
                out=den[:], lhsT=oneb[:],
                rhs=wcorr_s[:, band * BN : (band + 1) * BN],
                start=False, stop=True,
            )

        # final band's tail
        Pl = NBANDS - 1
        tail_feed(Pl)
        tail_recip(Pl)
        tail_out(Pl)

    nc.compile()
    return nc


def _bf(x):
    return np.ascontiguousarray(np.asarray(x, np.float32)).astype(BF_NP)


def make_in_maps(Q, K, V, Wq, bq, Wk, bk, Wv, bv):
    Q = np.asarray(Q, np.float32)
    K = np.asarray(K, np.float32)
    V = np.asarray(V, np.float32)
    G = np.asarray(Wq, np.float32) @ np.asarray(Wk, np.float32).T  # [C, C]
    gw = np.concatenate([G.T, np.asarray(Wv, np.float32)], axis=1)  # [C, C+OD]
    gwb = _bf(gw)

    # band mask constant, packed-layout [96, 1536]
    idx = np.arange(W)
    band96 = (np.abs(idx[:, None] - idx[None, :]) <= PAD).astype(np.float32)
    bandp = np.zeros((W, SPACK), np.float32)
    for i in PH_I[0]:
        o, w = OFF[i], WID[i]
        bandp[:, o : o + w * W] = np.tile(band96, (1, w))
    bandp = _bf(bandp)

    bw = (np.minimum(idx + PAD, W - 1) - np.maximum(idx - PAD, 0) + 1).astype(
        np.float32
    )  # valid kx count per x

    in_maps = []
    for core in range(NCORES):
        b = core // (H // ROWS)
        h_start = (core % (H // ROWS)) * ROWS

        qs = Q[b, h_start : h_start + ROWS]              # [24,96,128]
        qtc = _bf(qs.reshape(NQ, C).T)                   # [128,2304]

        kpad = np.zeros((KROWS, W, C), np.float32)
        vpad = np.zeros((KROWS, W, C), np.float32)
        for j in range(KROWS):
            gr = h_start - PAD + j
            if 0 <= gr < H:
                kpad[j] = K[b, gr]
                vpad[j] = V[b, gr]
        ktc = _bf(kpad.reshape(NK, C).T)                 # [128,2880]
        vtc = _bf(vpad.transpose(1, 0, 2))               # [96,30,128]

        wcorr = np.zeros((1, NBANDS * BN), np.float32)
        for band in range(NBANDS):
            for c in range(BAND):
                gr = h_start + band * BAND + c
                n_inv = sum(
                    1 for dy in range(-PAD, PAD + 1) if not (0 <= gr + dy < H)
                )
                if n_inv:
                    wcorr[0, band * BN + c * W : band * BN + (c + 1) * W] = -n_inv * bw
        in_maps.append(
            {
                "qt": qtc,
                "kt": ktc,
                "v": vtc,
                "gw": gwb,
                "bandp": bandp,
                "wcorr": _bf(wcorr),
            }
        )
    return in_maps


def gather(results):
    full = np.empty((B, H, W, OD), np.float32)
    for core in range(NCORES):
        b = core // (H // ROWS)
        h_start = (core % (H // ROWS)) * ROWS
        full[b, h_start : h_start + ROWS] = results[core]["out"]
    return full


_NC_CACHE = {}


def get_nc(path="v2"):
    if path not in _NC_CACHE:
        _NC_CACHE[path] = build_nc() if path == "v2" else build_nc_v1(
            with_bv=(path == "v1bv")
        )
    return _NC_CACHE[path]


def kernel(Q, K, V, Wq, bq, Wk, bk, Wv, bv):
    if np.any(np.asarray(bq)) or np.any(np.asarray(bk)):
        nc = get_nc("v1bv" if np.any(np.asarray(bv)) else "v1")
        in_maps = make_in_maps_v1(Q, K, V, Wq, bq, Wk, bk, Wv, bv)
    elif np.any(np.asarray(bv)):
        nc = get_nc("v1bv")
        in_maps = make_in_maps_v1(Q, K, V, Wq, bq, Wk, bk, Wv, bv)
    else:
        nc = get_nc("v2")
        in_maps = make_in_maps(Q, K, V, Wq, bq, Wk, bk, Wv, bv)
    res = run_bass_kernel_spmd(nc, in_maps, list(range(NCORES)))
    return gather(res.results)


# ======================================================================
# v1 fallback (original f32r kernel) — used only when a bias is nonzero.
# ======================================================================

WVN = 2 * OD
NEG = -30000.0


def build_nc_v1(with_bv=False):
    MDT = FR
    nc = bacc.Bacc(None, target_bir_lowering=False)
    qt = nc.dram_tensor("qt", [C, NQ], MDT, kind="ExternalInput")
    kt = nc.dram_tensor("kt", [C, NK], MDT, kind="ExternalInput")
    v = nc.dram_tensor("v", [W, KROWS, C], MDT, kind="ExternalInput")
    wq = nc.dram_tensor("wq", [C, KD], MDT, kind="ExternalInput")
    wk = nc.dram_tensor("wk", [C, KD], MDT, kind="ExternalInput")
    wv = nc.dram_tensor("wv", [C, WVN], MDT, kind="ExternalInput")
    bq = nc.dram_tensor("bq", [KD, 1], DT, kind="ExternalInput")
    bk = nc.dram_tensor("bk", [KD, 1], DT, kind="ExternalInput")
    bv = nc.dram_tensor("bv", [1, WVN], MDT, kind="ExternalInput")
    kbias = nc.dram_tensor("kbias", [W, KROWS], DT, kind="ExternalInput")
    ones_in = nc.dram_tensor("ones", [W, 1], MDT, kind="ExternalInput")
    b4 = nc.dram_tensor("b4", [W, NKR * BN], DT, kind="ExternalInput")
    out = nc.dram_tensor("out", [ROWS, W, OD], DT, kind="ExternalOutput")

    with tile.TileContext(nc) as tc, ExitStack() as ctx:
        consts = ctx.enter_context(tc.tile_pool(name="consts", bufs=1))
        slabs = ctx.enter_context(tc.tile_pool(name="slabs", bufs=1))
        e_pool = ctx.enter_context(tc.tile_pool(name="e_pool", bufs=3))
        o_pool = ctx.enter_context(tc.tile_pool(name="o_pool", bufs=2))
        r_pool = ctx.enter_context(tc.tile_pool(name="r_pool", bufs=2))
        rs_pool = ctx.enter_context(tc.tile_pool(name="rs_pool", bufs=8))
        outs = ctx.enter_context(tc.tile_pool(name="outs", bufs=3))
        ps_a = ctx.enter_context(tc.tile_pool(name="ps_a", bufs=3, space="PSUM"))
        ps_b = ctx.enter_context(tc.tile_pool(name="ps_b", bufs=2, space="PSUM"))
        ps_c = ctx.enter_context(tc.tile_pool(name="ps_c", bufs=2, space="PSUM"))

        wq_s = consts.tile([C, KD], MDT, tag="cw")
        nc.sync.dma_start(out=wq_s[:], in_=wq[:])
        wk_s = consts.tile([C, KD], MDT, tag="cw2")
        nc.sync.dma_start(out=wk_s[:], in_=wk[:])
        wv_s = consts.tile([C, WVN], MDT, tag="cw3")
        nc.sync.dma_start(out=wv_s[:], in_=wv[:])
        bq_s = consts.tile([KD, 1], DT, tag="cb")
        nc.sync.dma_start(out=bq_s[:], in_=bq[:])
        bk_s = consts.tile([KD, 1], DT, tag="cb2")
        nc.sync.dma_start(out=bk_s[:], in_=bk[:])
        kbias_s = consts.tile([W, KROWS], DT, tag="ckb")
        nc.sync.dma_start(out=kbias_s[:], in_=kbias[:])
        b4_s = consts.tile([W, NKR * BN], DT, tag="cb4")
        nc.sync.dma_start(out=b4_s[:], in_=b4[:])
        ones96 = consts.tile([W, 1], MDT, tag="cones")
        nc.sync.dma_start(out=ones96[:], in_=ones_in[:])
        ones1 = consts.tile([1, 1], DT, tag="cone1")
        nc.vector.memset(ones1[:], 1.0)
        if with_bv:
            bv_s = consts.tile([1, WVN], MDT, tag="cbv")
            nc.sync.dma_start(out=bv_s[:], in_=bv[:])

        qt_s = slabs.tile([C, NQ], MDT, tag="sqt")
        nc.sync.dma_start(out=qt_s[:], in_=qt[:])
        kt_s = slabs.tile([C, NK], MDT, tag="skt")
        nc.sync.dma_start(out=kt_s[:], in_=kt[:])
        v_s = slabs.tile([W, KROWS, C], MDT, tag="sv")
        nc.scalar.dma_start(out=v_s[:], in_=v[:])

        qT_s = slabs.tile([KD, NQ], MDT, tag="sqT")
        kT_s = slabs.tile([KD, NK], MDT, tag="skT")
        for dst, src, wmat, bvec, n in (
            (qT_s, qt_s, wq_s, bq_s, NQ),
            (kT_s, kt_s, wk_s, bk_s, NK),
        ):
            for j0 in range(0, n, 512):
                j1 = min(j0 + 512, n)
                ps = ps_a.tile([KD, 512], DT, tag="w")
                nc.tensor.matmul(
                    out=ps[:, : j1 - j0], lhsT=wmat[:], rhs=src[:, j0:j1],
                    start=True, stop=True,
                )
                nc.scalar.activation(
                    dst[:, j0:j1], ps[:, : j1 - j0], AF.Identity,
                    bias=bvec[:], scale=1.0,
                )

        for band in range(NBANDS):
            h0 = band * BAND
            jq = slice(h0 * W, (h0 + BAND) * W)
            outT = ps_b.tile([OD, BN], DT, tag="outT")
            den = ps_c.tile([1, BN], DT, tag="den")
            for i in range(NKR):
                r = h0 + i
                S = ps_a.tile([W, BN], DT, tag="w")
                nc.tensor.matmul(
                    out=S[:], lhsT=kT_s[:, r * W : (r + 1) * W],
                    rhs=qT_s[:, jq], start=True, stop=True,
                )
                E = e_pool.tile([W, BN], MDT, tag="E")
                nc.scalar.activation(
                    E[:], S[:], AF.Exp, bias=kbias_s[:, r : r + 1], scale=SCALE
                )
                nc.vector.tensor_mul(E[:], E[:], b4_s[:, i * BN : (i + 1) * BN])
                nc.tensor.matmul(
                    out=outT[:], lhsT=v_s[:, r, :], rhs=E[:],
                    start=(i == 0), stop=(i == NKR - 1),
                )
                nc.tensor.matmul(
                    out=den[:], lhsT=ones96[:], rhs=E[:],
                    start=(i == 0), stop=(i == NKR - 1),
                )

            recip = r_pool.tile([1, BN], DT, tag="recip")
            nc.vector.reciprocal(recip[:], den[:])
            oT = o_pool.tile([OD, BN], MDT, tag="oT")
            nc.vector.tensor_copy(oT[:], outT[:])
            if with_bv:
                den_sb = r_pool.tile([1, BN], MDT, tag="densb")
                nc.vector.tensor_copy(den_sb[:], den[:])
            for c in range(BAND):
                cs = slice(c * W, (c + 1) * W)
                rT = ps_a.tile([W, 1], DT, tag="w")
                nc.tensor.transpose(rT[:], recip[:, cs], ones1[:])
                rS = rs_pool.tile([W, 1], DT, tag="rS")
                nc.vector.tensor_copy(rS[:], rT[:])
                op = ps_a.tile([W, WVN], DT, tag="w")
                nc.tensor.matmul(
                    out=op[:], lhsT=oT[:, cs], rhs=wv_s[:],
                    start=True, stop=not with_bv,
                )
                if with_bv:
                    nc.tensor.matmul(
                        out=op[:], lhsT=den_sb[:, cs], rhs=bv_s[:],
                        start=False, stop=True,
                    )
                ost = outs.tile([W, OD], DT, tag="ost")
                nc.scalar.activation(ost[:], op[:, :OD], AF.Relu, bias=0.0, scale=rS[:])
                nc.sync.dma_start(out=out[h0 + c], in_=ost[:])

    nc.compile()
    return nc


def round_f32r(x):
    b = np.ascontiguousarray(x, np.float32).view(np.uint32)
    tie = (b >> 12) & 1
    b = (b + 0x7FF + tie) & np.uint32(0xFFFFF000)
    return b.view(np.float32)


def make_in_maps_v1(Q, K, V, Wq, bq, Wk, bk, Wv, bv):
    rnd = round_f32r
    Q = np.asarray(Q, np.float32)
    K = np.asarray(K, np.float32)
    V = np.asarray(V, np.float32)
    Wqr = rnd(np.asarray(Wq, np.float32))
    Wkr = rnd(np.asarray(Wk, np.float32))
    wvp = np.zeros((C, WVN), np.float32)
    wvp[:, :OD] = np.asarray(Wv, np.float32)
    wvp = rnd(wvp)
    bqv = np.ascontiguousarray(np.asarray(bq, np.float32).reshape(KD, 1))
    bkv = np.ascontiguousarray(np.asarray(bk, np.float32).reshape(KD, 1))
    bvp = np.zeros((1, WVN), np.float32)
    bvp[0, :OD] = np.asarray(bv, np.float32)
    bvp = rnd(bvp)

    idx = np.arange(W)
    b4 = (np.abs(idx[:, None] - idx[None, :]) <= PAD).astype(np.float32)
    b4i = np.zeros((W, NKR, BAND, W), np.float32)
    for i in range(NKR):
        for c in range(BAND):
            if i - 2 * PAD <= c <= i:
                b4i[:, i, c, :] = b4
    b4rep = np.ascontiguousarray(b4i.reshape(W, NKR * BAND * W))

    in_maps = []
    for core in range(NCORES):
        b = core // (H // ROWS)
        h_start = (core % (H // ROWS)) * ROWS
        qs = Q[b, h_start : h_start + ROWS]
        qtc = rnd(np.ascontiguousarray(qs.reshape(NQ, C).T))
        kpad = np.zeros((KROWS, W, C), np.float32)
        vpad = np.zeros((KROWS, W, C), np.float32)
        kb = np.full((KROWS,), NEG, np.float32)
        for j in range(KROWS):
            gr = h_start - PAD + j
            if 0 <= gr < H:
                kpad[j] = K[b, gr]
                vpad[j] = V[b, gr]
                kb[j] = 0.0
        ktc = rnd(np.ascontiguousarray(kpad.reshape(NK, C).T))
        vtc = rnd(np.ascontiguousarray(vpad.transpose(1, 0, 2)))
        kbias = np.ascontiguousarray(np.broadcast_to(kb[None, :], (W, KROWS)))
        in_maps.append(
            {
                "qt": qtc, "kt": ktc, "v": vtc,
                "wq": Wqr, "wk": Wkr, "wv": wvp,
                "bq": bqv, "bk": bkv, "bv": bvp,
                "kbias": kbias,
                "ones": np.ones((W, 1), np.float32),
                "b4": b4rep,
            }
        )
    return in_maps
